# revision 1
# baseline (speedup 1.0000x reference)
"""Trainium2 Bass kernel for nn_HCNetFull (dense_mlp), 8-core data parallel.

Strategy: shard the 32768 tokens across 8 NeuronCores (4096 each).
Token-major activations [128 tok, 512 feat] resident in SBUF; PE transposes
at matmul boundaries; geometric group mixing via per-group outer products
(DVE broadcast APs) + block-diagonal PE matmuls. All fp32.
"""

import numpy as np
from contextlib import ExitStack

import concourse.bass as bass
import concourse.tile as tile
from concourse import bacc, mybir
from concourse.bass_utils import run_bass_kernel_spmd
from concourse.masks import make_identity

F32 = mybir.dt.float32
D, DD, L, GS, G, P = 512, 1024, 8, 8, 64, 128
NCORES = 8
AF = mybir.ActivationFunctionType
ALU = None  # set lazily


def _alu():
    global ALU
    if ALU is None:
        ALU = mybir.AluOpType
    return ALU


def build_nc(T, CH, n2_affine):
    """Build the per-core Bass module for T tokens, chunk size CH."""
    alu = _alu()
    NT = T // P          # 128-token subtiles
    NCH = T // CH        # chunks
    TS = CH // P         # subtiles per chunk (4 for CH=512)

    nc = bacc.Bacc("TRN2", target_bir_lowering=False, debug=False)

    dram = {}
    def din(name, shape):
        dram[name] = nc.dram_tensor(name, list(shape), F32, kind="ExternalInput")
        return dram[name]

    xT = din("xT", (4, T))
    W1 = din("W1", (L, D, DD)); B1 = din("B1", (L, P, 8))
    W2 = din("W2", (L, DD, D)); B2 = din("B2", (L, P, 4))
    GEO = din("GEO", (L, 8, P, P)); GB = din("GB", (L, P, 4))
    WIN = din("WIN", (4, D)); BIN = din("BIN", (P, 4))
    GPV = din("GPV", (4, P, 16)); BPV = din("BPV", (16, 1))
    GIW = din("GIW", (G, D)); BGI = din("BGI", (P, 4))
    PI1 = din("PI1", (D, D)); BP1 = din("BP1", (P, 4))
    PI2 = din("PI2", (D, D)); BP2 = din("BP2", (P, 4))
    OW = din("OW", (4, P, 4)); OB = din("OB", (4, 1))
    if n2_affine:
        G2R = din("G2R", (L, P, D)); B2R = din("B2R", (L, P, D))
    OUT = nc.dram_tensor("OUT", [4, T], F32, kind="ExternalOutput")

    with tile.TileContext(nc) as tc, ExitStack() as _px:
        cst = _px.enter_context(tc.tile_pool(name="cst", bufs=1))
        wl = _px.enter_context(tc.tile_pool(name="wl", bufs=1))
        hp = _px.enter_context(tc.tile_pool(name="hp", bufs=1))
        act = _px.enter_context(tc.tile_pool(name="act", bufs=1))
        pp = _px.enter_context(tc.tile_pool(name="pp", bufs=1))
        sm = _px.enter_context(tc.tile_pool(name="sm", bufs=2))
        st = _px.enter_context(tc.tile_pool(name="st", bufs=8))
        ps_mm = _px.enter_context(tc.tile_pool(name="ps_mm", bufs=2, space="PSUM"))
        ps_tp = _px.enter_context(tc.tile_pool(name="ps_tp", bufs=2, space="PSUM"))
        ps_g = _px.enter_context(tc.tile_pool(name="ps_g", bufs=1, space="PSUM"))
        ps_s = _px.enter_context(tc.tile_pool(name="ps_s", bufs=1, space="PSUM"))

        ident = cst.tile([P, P], F32)
        make_identity(nc, ident)
        eps_t = cst.tile([P, 1], F32)
        nc.vector.memset(eps_t, 1e-5)
        win_sb = cst.tile([4, 4, P], F32)
        nc.sync.dma_start(out=win_sb, in_=WIN[:, :].rearrange("p (mt c) -> p mt c", c=P))
        bin_sb = cst.tile([P, 4], F32)
        nc.sync.dma_start(out=bin_sb, in_=BIN[:, :])
        gpv_sb = cst.tile([P, 4, 16], F32)
        nc.sync.dma_start(out=gpv_sb, in_=GPV[:, :, :].rearrange("kt p c -> p kt c"))
        bpv_sb = cst.tile([16, 1], F32)
        nc.sync.dma_start(out=bpv_sb, in_=BPV[:, :])
        bgi_sb = cst.tile([P, 4], F32)
        nc.sync.dma_start(out=bgi_sb, in_=BGI[:, :])
        bp1_sb = cst.tile([P, 4], F32)
        nc.sync.dma_start(out=bp1_sb, in_=BP1[:, :])
        bp2_sb = cst.tile([P, 4], F32)
        nc.sync.dma_start(out=bp2_sb, in_=BP2[:, :])
        ow_sb = cst.tile([P, 4, 4], F32)
        nc.sync.dma_start(out=ow_sb, in_=OW[:, :, :].rearrange("kt p c -> p kt c"))
        ob_sb = cst.tile([4, 1], F32)
        nc.sync.dma_start(out=ob_sb, in_=OB[:, :])

        h_sb = hp.tile([P, NT, D], F32)

        def ln_stats(src):
            s6 = st.tile([P, 6], F32, tag="s6")
            nc.vector.bn_stats(out=s6, in_=src)
            mv = st.tile([P, 2], F32, tag="mv")
            nc.vector.bn_aggr(out=mv, in_=s6)
            sd = st.tile([P, 1], F32, tag="sd")
            nc.scalar.activation(out=sd, in_=mv[:, 1:2], func=AF.Sqrt, bias=eps_t)
            rs = st.tile([P, 1], F32, tag="rs")
            nc.vector.reciprocal(out=rs, in_=sd)
            return mv, rs

        def transpose_in(src4, dst, tagp="tpb"):
            """src4: fn(ts)->AP [128 tok,128 f]; dst [128 f, CH tok] sbuf (or None->psum)"""
            tpb = ps_tp.tile([P, CH], F32, tag=tagp)
            for ts in range(TS):
                nc.tensor.transpose(tpb[:, ts * P:(ts + 1) * P], src4(ts), ident)
            if dst is not None:
                nc.scalar.copy(out=dst, in_=tpb)
            return tpb

        # ---- input projection: h0 = x @ Win + bin ----
        for c in range(NCH):
            xc = sm.tile([4, CH], F32, tag="xc")
            nc.sync.dma_start(out=xc, in_=xT[:, c * CH:(c + 1) * CH])
            for mt in range(4):
                pm = ps_mm.tile([P, CH], F32, tag="mm")
                nc.tensor.matmul(pm, win_sb[:, mt, :], xc, start=True, stop=True)
                h0f = sm.tile([P, CH], F32, tag="h0f")
                nc.scalar.activation(out=h0f, in_=pm, func=AF.Identity,
                                     bias=bin_sb[:, mt:mt + 1])
                tpb = ps_tp.tile([P, CH], F32, tag="tpb")
                for ts in range(TS):
                    nc.tensor.transpose(tpb[:, ts * P:(ts + 1) * P],
                                        h0f[:, ts * P:(ts + 1) * P], ident)
                nc.scalar.copy(
                    out=h_sb[:, c * TS:(c + 1) * TS, mt * P:(mt + 1) * P],
                    in_=tpb.rearrange("p (ts c) -> p ts c", c=P))

        # ---- transformer layers ----
        for l in range(L):
            w1t = wl.tile([P, 4, DD], F32, tag="w1")
            nc.sync.dma_start(out=w1t, in_=W1[l].rearrange("(kt p) c -> p kt c", p=P))
            w2t = wl.tile([P, 8, D], F32, tag="w2")
            nc.sync.dma_start(out=w2t, in_=W2[l].rearrange("(kt p) c -> p kt c", p=P))
            geot = wl.tile([P, 8, P], F32, tag="geo")
            nc.sync.dma_start(out=geot, in_=GEO[l].rearrange("kp p c -> p kp c"))
            b1t = wl.tile([P, 8], F32, tag="b1")
            nc.sync.dma_start(out=b1t, in_=B1[l])
            b2t = wl.tile([P, 4], F32, tag="b2")
            nc.sync.dma_start(out=b2t, in_=B2[l])
            gbt = wl.tile([P, 4], F32, tag="gb")
            nc.sync.dma_start(out=gbt, in_=GB[l])
            if n2_affine:
                g2t = wl.tile([P, D], F32, tag="g2")
                nc.sync.dma_start(out=g2t, in_=G2R[l])
                b2rt = wl.tile([P, D], F32, tag="b2r")
                nc.sync.dma_start(out=b2rt, in_=B2R[l])

            for c in range(NCH):
                st0 = c * TS
                # LN1 (no affine: absorbed into W1/B1 host-side)
                xln = act.tile([P, TS, D], F32, tag="bufA")
                for ts in range(TS):
                    mv, rs = ln_stats(h_sb[:, st0 + ts, :])
                    nc.vector.tensor_scalar(
                        out=xln[:, ts, :], in0=h_sb[:, st0 + ts, :],
                        scalar1=mv[:, 0:1], scalar2=rs,
                        op0=alu.subtract, op1=alu.mult)
                # transpose -> feature-major rhs
                xTf = act.tile([P, 4, CH], F32, tag="xTf")
                for ft in range(4):
                    transpose_in(lambda ts: xln[:, ts, ft * P:(ft + 1) * P],
                                 xTf[:, ft, :])
                # fc1 + gelu
                z1 = act.tile([P, 8, CH], F32, tag="z1")
                for mt in range(8):
                    pm = ps_mm.tile([P, CH], F32, tag="mm")
                    for kt in range(4):
                        nc.tensor.matmul(pm, w1t[:, kt, mt * P:(mt + 1) * P],
                                         xTf[:, kt, :], start=(kt == 0), stop=(kt == 3))
                    nc.scalar.activation(out=z1[:, mt, :], in_=pm, func=AF.Gelu,
                                         bias=b1t[:, mt:mt + 1])
                # fc2
                z2 = act.tile([P, 4, CH], F32, tag="bufA")
                for ft in range(4):
                    pm = ps_mm.tile([P, CH], F32, tag="mm")
                    for kt in range(8):
                        nc.tensor.matmul(pm, w2t[:, kt, ft * P:(ft + 1) * P],
                                         z1[:, kt, :], start=(kt == 0), stop=(kt == 7))
                    nc.scalar.activation(out=z2[:, ft, :], in_=pm, func=AF.Identity,
                                         bias=b2t[:, ft:ft + 1])
                # transpose back + residual
                y = act.tile([P, TS, D], F32, tag="y")
                for ts in range(TS):
                    tpb = transpose_in(
                        lambda ft: z2[:, ft, ts * P:(ts + 1) * P], None)
                    # NOTE: src4 indexes ft here (4 feature blocks of this ts)
                    nc.vector.tensor_add(out=y[:, ts, :], in0=tpb,
                                         in1=h_sb[:, st0 + ts, :])
                # geometric mixing
                for ts in range(TS):
                    Pt = pp.tile([P, G, GS, GS], F32, tag="P")
                    a = y[:, ts, :].rearrange("p (g i) -> p g i", i=GS)
                    nc.vector.tensor_mul(
                        out=Pt,
                        in0=a.unsqueeze(3).to_broadcast((P, G, GS, GS)),
                        in1=a.unsqueeze(2).to_broadcast((P, G, GS, GS)))
                    Pf = Pt.rearrange("p g i j -> p (g i j)")
                    gsb = sm.tile([P, 4, P], F32, tag="gsb")
                    for mt in range(4):
                        pg = ps_g.tile([P, P], F32, tag="gps")
                        for kh in range(2):
                            tp2 = ps_tp.tile([P, CH], F32, tag="tp2")
                            for q in range(4):
                                kk = mt * 8 + kh * 4 + q
                                nc.tensor.transpose(
                                    tp2[:, q * P:(q + 1) * P],
                                    Pf[:, kk * P:(kk + 1) * P], ident)
                            rhs4 = sm.tile([P, CH], F32, tag="rhs4")
                            nc.vector.tensor_copy(out=rhs4, in_=tp2)
                            for q in range(4):
                                kp = kh * 4 + q
                                nc.tensor.matmul(
                                    pg, geot[:, kp, :], rhs4[:, q * P:(q + 1) * P],
                                    start=(kp == 0), stop=(kp == 7))
                        nc.scalar.activation(out=gsb[:, mt, :], in_=pg,
                                             func=AF.Identity, bias=gbt[:, mt:mt + 1])
                    tpb = transpose_in(lambda mt: gsb[:, mt, ts * 0:P], None)
                    # ^ gsb[:, mt, :] is [128 geo-feat, 128 tok of this ts]
                    nc.vector.scalar_tensor_tensor(
                        out=y[:, ts, :], in0=tpb, scalar=0.1, in1=y[:, ts, :],
                        op0=alu.mult, op1=alu.add)
                # LN2 -> h
                for ts in range(TS):
                    mv, rs = ln_stats(y[:, ts, :])
                    nc.vector.tensor_scalar(
                        out=h_sb[:, st0 + ts, :], in0=y[:, ts, :],
                        scalar1=mv[:, 0:1], scalar2=rs,
                        op0=alu.subtract, op1=alu.mult)
                    if n2_affine:
                        nc.vector.tensor_mul(out=h_sb[:, st0 + ts, :],
                                             in0=h_sb[:, st0 + ts, :], in1=g2t)
                        nc.vector.tensor_add(out=h_sb[:, st0 + ts, :],
                                             in0=h_sb[:, st0 + ts, :], in1=b2rt)

        # ---- GeometricInteraction ----
        giw_sb = wl.tile([G, D], F32, tag="geo")
        nc.sync.dma_start(out=giw_sb, in_=GIW[:, :])
        pi1_sb = wl.tile([P, 4, D], F32, tag="w1")
        nc.sync.dma_start(out=pi1_sb, in_=PI1[:, :].rearrange("(kt p) c -> p kt c", p=P))
        pi2_sb = wl.tile([P, 4, D], F32, tag="w2")
        nc.sync.dma_start(out=pi2_sb, in_=PI2[:, :].rearrange("(kt p) c -> p kt c", p=P))
        for c in range(NCH):
            st0 = c * TS
            hTf = act.tile([P, 4, CH], F32, tag="xTf")
            for ft in range(4):
                transpose_in(lambda ts: h_sb[:, st0 + ts, ft * P:(ft + 1) * P],
                             hTf[:, ft, :])
            pv = ps_s.tile([16, CH], F32, tag="sps")
            for kt in range(4):
                nc.tensor.matmul(pv, gpv_sb[:, kt, :], hTf[:, kt, :],
                                 start=(kt == 0), stop=(kt == 3))
            pvsb = sm.tile([16, CH], F32, tag="pvsb")
            nc.scalar.activation(out=pvsb, in_=pv, func=AF.Identity, bias=bpv_sb)
            ivT = sm.tile([G, TS, P], F32, tag="ivT")
            for ts in range(TS):
                tp2 = ps_tp.tile([P, CH], F32, tag="tp2")
                nc.tensor.transpose(tp2[:, 0:16], pvsb[:, ts * P:(ts + 1) * P],
                                    ident[:16, :16])
                pvt = sm.tile([P, 16], F32, tag="pvt")
                nc.vector.tensor_copy(out=pvt, in_=tp2[:, 0:16])
                iv = sm.tile([P, GS, GS], F32, tag="iv")
                nc.vector.tensor_mul(
                    out=iv,
                    in0=pvt[:, 0:8].unsqueeze(2).to_broadcast((P, GS, GS)),
                    in1=pvt[:, 8:16].unsqueeze(1).to_broadcast((P, GS, GS)))
                tp3 = ps_tp.tile([P, CH], F32, tag="tpb")
                nc.tensor.transpose(tp3[:G, 0:P], iv.rearrange("p a b -> p (a b)"),
                                    ident)
                nc.vector.tensor_copy(out=ivT[:, ts, :], in_=tp3[:G, 0:P])
            z2 = act.tile([P, 4, CH], F32, tag="bufA")
            for ft in range(4):
                pm = ps_mm.tile([P, CH], F32, tag="mm")
                nc.tensor.matmul(pm, giw_sb[:, ft * P:(ft + 1) * P],
                                 ivT.rearrange("p ts c -> p (ts c)"),
                                 start=True, stop=True)
                nc.scalar.activation(out=z2[:, ft, :], in_=pm, func=AF.Identity,
                                     bias=bgi_sb[:, ft:ft + 1])
            y = act.tile([P, TS, D], F32, tag="y")
            for ts in range(TS):
                tpb = transpose_in(lambda ft: z2[:, ft, ts * P:(ts + 1) * P], None)
                nc.vector.tensor_add(out=y[:, ts, :], in0=tpb,
                                     in1=h_sb[:, st0 + ts, :])
            for ts in range(TS):
                mv, rs = ln_stats(y[:, ts, :])
                nc.vector.tensor_scalar(
                    out=h_sb[:, st0 + ts, :], in0=y[:, ts, :],
                    scalar1=mv[:, 0:1], scalar2=rs,
                    op0=alu.subtract, op1=alu.mult)

        # ---- particle MLP + output ----
        for c in range(NCH):
            st0 = c * TS
            hTf = act.tile([P, 4, CH], F32, tag="xTf")
            for ft in range(4):
                transpose_in(lambda ts: h_sb[:, st0 + ts, ft * P:(ft + 1) * P],
                             hTf[:, ft, :])
            z1 = act.tile([P, 8, CH], F32, tag="z1")
            for mt in range(4):
                pm = ps_mm.tile([P, CH], F32, tag="mm")
                for kt in range(4):
                    nc.tensor.matmul(pm, pi1_sb[:, kt, mt * P:(mt + 1) * P],
                                     hTf[:, kt, :], start=(kt == 0), stop=(kt == 3))
                nc.scalar.activation(out=z1[:, mt, :], in_=pm, func=AF.Gelu,
                                     bias=bp1_sb[:, mt:mt + 1])
            z2 = act.tile([P, 4, CH], F32, tag="bufA")
            for ft in range(4):
                pm = ps_mm.tile([P, CH], F32, tag="mm")
                for kt in range(4):
                    nc.tensor.matmul(pm, pi2_sb[:, kt, ft * P:(ft + 1) * P],
                                     z1[:, kt, :], start=(kt == 0), stop=(kt == 3))
                nc.scalar.activation(out=z2[:, ft, :], in_=pm, func=AF.Identity,
                                     bias=bp2_sb[:, ft:ft + 1])
            po = ps_s.tile([16, CH], F32, tag="sps")
            for kt in range(4):
                nc.tensor.matmul(po[:4, :], ow_sb[:, kt, :], z2[:, kt, :],
                                 start=(kt == 0), stop=(kt == 3))
            xc = sm.tile([4, CH], F32, tag="xc")
            nc.sync.dma_start(out=xc, in_=xT[:, c * CH:(c + 1) * CH])
            osb = sm.tile([4, CH], F32, tag="osb")
            nc.vector.scalar_tensor_tensor(
                out=osb, in0=po[:4, :], scalar=ob_sb, in1=xc,
                op0=alu.add, op1=alu.add)
            nc.sync.dma_start(out=OUT[:, c * CH:(c + 1) * CH], in_=osb)

    nc.compile()
    return nc


def _prepack(inputs, T):
    """Host-side weight packing (fp32 numpy)."""
    f = lambda a: np.ascontiguousarray(np.asarray(a, np.float32))
    x = f(inputs["x"]).reshape(-1, 4)
    in_w, in_b = f(inputs["in_w"]), f(inputs["in_b"])
    fc1_w, fc1_b = f(inputs["fc1_w"]), f(inputs["fc1_b"])
    fc2_w, fc2_b = f(inputs["fc2_w"]), f(inputs["fc2_b"])
    geo_w, geo_b = f(inputs["geo_w"]), f(inputs["geo_b"])
    n1_g, n1_b = f(inputs["n1_g"]), f(inputs["n1_b"])
    n2_g, n2_b = f(inputs["n2_g"]), f(inputs["n2_b"])

    W1 = n1_g[:, :, None] * fc1_w                      # [L,512,1024]
    b1full = fc1_b + np.einsum("ld,lde->le", n1_b, fc1_w)
    B1 = b1full.reshape(L, 8, P).transpose(0, 2, 1).copy()
    W2 = fc2_w
    B2 = fc2_b.reshape(L, 4, P).transpose(0, 2, 1).copy()
    GEO = np.zeros((L, 8, P, P), np.float32)
    for l in range(L):
        gw2 = geo_w[l]                                  # [64, 8]
        for kp in range(8):
            for gp in range(2):
                c0 = (2 * kp + gp) * 8
                GEO[l, kp, gp * G:(gp + 1) * G, c0:c0 + 8] = gw2
    gbfull = np.tile(geo_b, (1, G))                     # [L, 512]
    GB = gbfull.reshape(L, 4, P).transpose(0, 2, 1).copy()
    BIN = in_b.reshape(4, P).T.copy()
    GPV = np.concatenate(
        [f(inputs["gi_pos_w"]), f(inputs["gi_vel_w"])], axis=1
    ).reshape(4, P, 16).copy()
    BPV = np.concatenate([f(inputs["gi_pos_b"]), f(inputs["gi_vel_b"])])[:, None]
    GIW = f(inputs["gi_int_w"])
    BGI = f(inputs["gi_int_b"]).reshape(4, P).T.copy()
    gn_g, gn_b = f(inputs["gi_n_g"]), f(inputs["gi_n_b"])
    PI1 = gn_g[:, None] * f(inputs["pi1_w"])
    bp1full = f(inputs["pi1_b"]) + gn_b @ f(inputs["pi1_w"])
    BP1 = bp1full.reshape(4, P).T.copy()
    PI2 = f(inputs["pi2_w"])
    BP2 = f(inputs["pi2_b"]).reshape(4, P).T.copy()
    OW = f(inputs["out_w"]).reshape(4, P, 4).copy()
    OB = f(inputs["out_b"])[:, None]

    n2_affine = not (np.all(n2_g == 1.0) and np.all(n2_b == 0.0))
    shared = dict(W1=W1, B1=B1, W2=W2, B2=B2, GEO=GEO, GB=GB,
                  WIN=in_w, BIN=BIN, GPV=GPV, BPV=BPV, GIW=GIW, BGI=BGI,
                  PI1=PI1, BP1=BP1, PI2=PI2, BP2=BP2, OW=OW, OB=OB)
    if n2_affine:
        shared["G2R"] = np.ascontiguousarray(
            np.broadcast_to(n2_g[:, None, :], (L, P, D)), np.float32)
        shared["B2R"] = np.ascontiguousarray(
            np.broadcast_to(n2_b[:, None, :], (L, P, D)), np.float32)
    shared = {k: np.ascontiguousarray(v, np.float32) for k, v in shared.items()}

    in_maps = []
    for c in range(NCORES):
        m = dict(shared)
        m["xT"] = np.ascontiguousarray(x[c * T:(c + 1) * T].T)
        in_maps.append(m)
    return in_maps, n2_affine


_CACHE = {}


def _get_compiled(T, CH, n2_affine):
    key = (T, CH, n2_affine)
    if key not in _CACHE:
        _CACHE[key] = build_nc(T, CH, n2_affine)
    return _CACHE[key]


def kernel(**inputs):
    x = np.asarray(inputs["x"])
    B, N, _ = x.shape
    T = B * N // NCORES
    in_maps, n2_affine = _prepack(inputs, T)
    nc = _get_compiled(T, 512, n2_affine)
    res = run_bass_kernel_spmd(nc, in_maps, core_ids=list(range(NCORES)))
    outs = [res.results[c]["OUT"].T for c in range(NCORES)]   # [T,4] each
    full = np.concatenate(outs, axis=0).reshape(B, N, 4).astype(np.float32)
    return full



# revision 5
# speedup vs baseline: 32.3881x; 32.3881x over previous
"""Trainium2 Bass kernel for nn_HCNetFull (dense_mlp), 8-core data parallel.

Strategy: shard the 32768 tokens across 8 NeuronCores (4096 each).
Token-major bf16 activations resident in SBUF; all matmuls/transposes in
bf16 (PE 1 cyc/col vs 4 for fp32); fp32 accumulation in PSUM and fp32
LayerNorm statistics.  The per-group outer-product mixing uses the
modular-shift symmetric factorization: x_i*x_j terms are covered by the
40 products x_i * x_{(i+d)%8}, d=0..4, so the contraction is a 2560->512
block matmul whose 128x128 lhsT chunks repeat with period 5.
LN1 of layers >=1 is skipped: its input is the previous LN2 output
(already zero-mean/unit-var, and n1_g=1, n1_b=0), so LN1 is an identity
up to O(eps)=1e-5.

Host side: the jitted shard_map executable and the device-resident
weight arrays are cached across kernel() calls (the axon tunnel moves
~40MB/s, so re-shipping 130MB of replicated weights per call dominates
wall time otherwise).  Weights are revalidated by byte comparison
against the cached host copies each call.
"""

import numpy as np
from contextlib import ExitStack

import concourse.bass as bass
import concourse.tile as tile
from concourse import bacc, mybir
from concourse.bass_utils import run_bass_kernel_spmd
from concourse.masks import make_identity

F32 = mybir.dt.float32
BF16 = mybir.dt.bfloat16
NPBF = mybir.dt.np(BF16)
D, DD, L, GS, G, P = 512, 1024, 8, 8, 64, 128
NCORES = 8
AF = mybir.ActivationFunctionType
ALU = None


def _alu():
    global ALU
    if ALU is None:
        ALU = mybir.AluOpType
    return ALU


def build_nc(T, CH, ln_skip):
    alu = _alu()
    NT = T // P
    NCH = T // CH
    TS = CH // P

    nc = bacc.Bacc("TRN2", target_bir_lowering=False, debug=False)

    def din(name, shape, dt=BF16):
        return nc.dram_tensor(name, list(shape), dt, kind="ExternalInput")

    xT = din("xT", (4, T), F32)
    XB = din("XB", (4, T))
    W1 = din("W1", (L, D, DD)); B1 = din("B1", (L, P, 8), F32)
    W2 = din("W2", (L, DD, D)); B2 = din("B2", (L, P, 4), F32)
    GEOS = din("GEOS", (L, 5, P, P)); GBT = din("GBT", (L, P, 1), F32)
    WIN = din("WIN", (4, D)); BIN = din("BIN", (P, 4), F32)
    GPV = din("GPV", (4, P, 16)); BPV = din("BPV", (16, 1), F32)
    GIW = din("GIW", (G, D)); BGI = din("BGI", (P, 4), F32)
    PI1 = din("PI1", (D, D)); BP1 = din("BP1", (P, 4), F32)
    PI2 = din("PI2", (D, D)); BP2 = din("BP2", (P, 4), F32)
    OW = din("OW", (4, P, 4)); OB = din("OB", (4, 1), F32)
    if not ln_skip:
        G2R = din("G2R", (L, P, D), F32); B2R = din("B2R", (L, P, D), F32)
    OUT = nc.dram_tensor("OUT", [4, T], F32, kind="ExternalOutput")

    with tile.TileContext(nc) as tc, ExitStack() as _px:
        cst = _px.enter_context(tc.tile_pool(name="cst", bufs=1))
        wl = _px.enter_context(tc.tile_pool(name="wl", bufs=2))
        hp = _px.enter_context(tc.tile_pool(name="hp", bufs=1))
        xfp = _px.enter_context(tc.tile_pool(name="xfp", bufs=2))
        z1p = _px.enter_context(tc.tile_pool(name="z1p", bufs=2))
        z2p = _px.enter_context(tc.tile_pool(name="z2p", bufs=2))
        yp = _px.enter_context(tc.tile_pool(name="yp", bufs=2))
        yep = _px.enter_context(tc.tile_pool(name="yep", bufs=1))
        pp = _px.enter_context(tc.tile_pool(name="pp", bufs=1))
        rp = _px.enter_context(tc.tile_pool(name="rp", bufs=2))
        gfp = _px.enter_context(tc.tile_pool(name="gfp", bufs=2))
        sm = _px.enter_context(tc.tile_pool(name="sm", bufs=2))
        st = _px.enter_context(tc.tile_pool(name="st", bufs=8))
        ps_tp = _px.enter_context(tc.tile_pool(name="ps_tp", bufs=2, space="PSUM"))
        ps_mm = _px.enter_context(tc.tile_pool(name="ps_mm", bufs=2, space="PSUM"))
        ps_g = _px.enter_context(tc.tile_pool(name="ps_g", bufs=2, space="PSUM"))
        ps_s = _px.enter_context(tc.tile_pool(name="ps_s", bufs=1, space="PSUM"))

        ident = cst.tile([P, P], BF16)
        make_identity(nc, ident)
        eps_t = cst.tile([P, 1], F32)
        nc.vector.memset(eps_t, 1e-5)
        win_sb = cst.tile([4, 4, P], BF16)
        nc.sync.dma_start(out=win_sb, in_=WIN[:, :].rearrange("p (mt c) -> p mt c", c=P))
        bin_sb = cst.tile([P, 4], F32)
        nc.sync.dma_start(out=bin_sb, in_=BIN[:, :])
        gpv_sb = cst.tile([P, 4, 16], BF16)
        nc.sync.dma_start(out=gpv_sb, in_=GPV[:, :, :].rearrange("kt p c -> p kt c"))
        bpv_sb = cst.tile([16, 1], F32)
        nc.sync.dma_start(out=bpv_sb, in_=BPV[:, :])
        giw_sb = cst.tile([G, D], BF16)
        nc.sync.dma_start(out=giw_sb, in_=GIW[:, :])
        bgi_sb = cst.tile([P, 4], F32)
        nc.sync.dma_start(out=bgi_sb, in_=BGI[:, :])
        pi1_sb = cst.tile([P, 4, D], BF16)
        nc.sync.dma_start(out=pi1_sb, in_=PI1[:, :].rearrange("(kt p) c -> p kt c", p=P))
        pi2_sb = cst.tile([P, 4, D], BF16)
        nc.sync.dma_start(out=pi2_sb, in_=PI2[:, :].rearrange("(kt p) c -> p kt c", p=P))
        bp1_sb = cst.tile([P, 4], F32)
        nc.sync.dma_start(out=bp1_sb, in_=BP1[:, :])
        bp2_sb = cst.tile([P, 4], F32)
        nc.sync.dma_start(out=bp2_sb, in_=BP2[:, :])
        ow_sb = cst.tile([P, 4, 4], BF16)
        nc.sync.dma_start(out=ow_sb, in_=OW[:, :, :].rearrange("kt p c -> p kt c"))
        ob_sb = cst.tile([4, 1], F32)
        nc.sync.dma_start(out=ob_sb, in_=OB[:, :])

        h_sb = hp.tile([P, NT, D], BF16)

        def ln_stats(src):
            s6 = st.tile([P, 6], F32, tag="s6")
            nc.vector.bn_stats(out=s6, in_=src)
            mv = st.tile([P, 2], F32, tag="mv")
            nc.vector.bn_aggr(out=mv, in_=s6)
            sd = st.tile([P, 1], F32, tag="sd")
            nc.scalar.activation(out=sd, in_=mv[:, 1:2], func=AF.Sqrt, bias=eps_t)
            rs = st.tile([P, 1], F32, tag="rs")
            nc.vector.reciprocal(out=rs, in_=sd)
            return mv, rs

        # rotate PSUM->SBUF copies (GPSIMD cannot access PSUM)
        cp_engines = [nc.vector, nc.scalar]
        cp_i = [0]

        def cp(out, in_):
            e = cp_engines[cp_i[0] % len(cp_engines)]
            cp_i[0] += 1
            if e is nc.scalar:
                e.copy(out=out, in_=in_)
            else:
                e.tensor_copy(out=out, in_=in_)

        def tr_feat(src_ts, dst):
            """src_ts(ts)->AP [128 tok,128 f]; dst [128 f, CH tok] sbuf."""
            tpb = ps_tp.tile([P, CH], BF16, tag="tp")
            for ts in range(TS):
                nc.tensor.transpose(tpb[:, ts * P:(ts + 1) * P], src_ts(ts), ident)
            cp(dst, tpb)

        # ---- input projection: h0 = x @ Win + bin (token-major bf16) ----
        for c in range(NCH):
            xcb = sm.tile([4, CH], BF16, tag="xcb")
            nc.sync.dma_start(out=xcb, in_=XB[:, c * CH:(c + 1) * CH])
            h0f = sm.tile([P, 4, CH], BF16, tag="h0f")
            for mt in range(4):
                pm = ps_mm.tile([P, CH], F32, tag="mm")
                nc.tensor.matmul(pm, win_sb[:, mt, :], xcb, start=True, stop=True)
                nc.scalar.activation(out=h0f[:, mt, :], in_=pm, func=AF.Identity,
                                     bias=bin_sb[:, mt:mt + 1])
            for ts in range(TS):
                tpb = ps_tp.tile([P, CH], BF16, tag="tp")
                for ft in range(4):
                    nc.tensor.transpose(tpb[:, ft * P:(ft + 1) * P],
                                        h0f[:, ft, ts * P:(ts + 1) * P], ident)
                cp(h_sb[:, c * TS + ts, :], tpb)

        # ---- transformer layers ----
        for l in range(L):
            w1t = wl.tile([P, 4, DD], BF16, tag="w1")
            nc.sync.dma_start(out=w1t, in_=W1[l].rearrange("(kt p) c -> p kt c", p=P))
            w2t = wl.tile([P, 8, D], BF16, tag="w2")
            nc.sync.dma_start(out=w2t, in_=W2[l].rearrange("(kt p) c -> p kt c", p=P))
            geot = wl.tile([P, 5, P], BF16, tag="geo")
            nc.sync.dma_start(out=geot, in_=GEOS[l].rearrange("r p c -> p r c"))
            b1t = wl.tile([P, 8], F32, tag="b1")
            nc.sync.dma_start(out=b1t, in_=B1[l])
            b2t = wl.tile([P, 4], F32, tag="b2")
            nc.sync.dma_start(out=b2t, in_=B2[l])
            gbt = wl.tile([P, 1], F32, tag="gb")
            nc.sync.dma_start(out=gbt, in_=GBT[l])
            if not ln_skip:
                g2t = wl.tile([P, D], F32, tag="g2")
                nc.sync.dma_start(out=g2t, in_=G2R[l])
                b2rt = wl.tile([P, D], F32, tag="b2r")
                nc.sync.dma_start(out=b2rt, in_=B2R[l])

            for c in range(NCH):
                st0 = c * TS
                # LN1: identity for l>=1 in the ln_skip regime
                if l == 0 or not ln_skip:
                    xln = xfp.tile([P, TS, D], BF16, tag="xln")
                    for ts in range(TS):
                        mv, rs = ln_stats(h_sb[:, st0 + ts, :])
                        nc.vector.tensor_scalar(
                            out=xln[:, ts, :], in0=h_sb[:, st0 + ts, :],
                            scalar1=mv[:, 0:1], scalar2=rs,
                            op0=alu.subtract, op1=alu.mult)
                    src = lambda ts, f0: xln[:, ts, f0:f0 + P]
                else:
                    src = lambda ts, f0: h_sb[:, st0 + ts, f0:f0 + P]
                xtf = xfp.tile([P, 4, CH], BF16, tag="xtf")
                for ft in range(4):
                    tr_feat(lambda ts: src(ts, ft * P), xtf[:, ft, :])
                # fc1 + gelu
                z1 = z1p.tile([P, 8, CH], BF16, tag="z1")
                for mt in range(8):
                    pm = ps_mm.tile([P, CH], F32, tag="mm")
                    for kt in range(4):
                        nc.tensor.matmul(pm, w1t[:, kt, mt * P:(mt + 1) * P],
                                         xtf[:, kt, :], start=(kt == 0), stop=(kt == 3))
                    nc.scalar.activation(out=z1[:, mt, :], in_=pm, func=AF.Gelu,
                                         bias=b1t[:, mt:mt + 1])
                # fc2
                z2 = z2p.tile([P, 4, CH], BF16, tag="z2")
                for ft in range(4):
                    pm = ps_mm.tile([P, CH], F32, tag="mm")
                    for kt in range(8):
                        nc.tensor.matmul(pm, w2t[:, kt, ft * P:(ft + 1) * P],
                                         z1[:, kt, :], start=(kt == 0), stop=(kt == 7))
                    nc.scalar.activation(out=z2[:, ft, :], in_=pm, func=AF.Identity,
                                         bias=b2t[:, ft:ft + 1])
                # transpose back + residual (fp32 y)
                y = yp.tile([P, TS, D], F32, tag="y")
                for ts in range(TS):
                    tpb = ps_tp.tile([P, CH], BF16, tag="tp")
                    for ft in range(4):
                        nc.tensor.transpose(tpb[:, ft * P:(ft + 1) * P],
                                            z2[:, ft, ts * P:(ts + 1) * P], ident)
                    nc.vector.tensor_add(out=y[:, ts, :], in0=tpb,
                                         in1=h_sb[:, st0 + ts, :])
                # geometric mixing: P features f = 40g + 8d + i,
                # P[g,d,i] = y[g,i] * y[g,(i+d)%8]
                yx = yep.tile([P, TS, G, 16], BF16, tag="yx")
                y4 = y.rearrange("p ts (g i) -> p ts g i", i=GS)
                nc.gpsimd.tensor_copy(out=yx[:, :, :, 0:8], in_=y4)
                nc.gpsimd.tensor_copy(out=yx[:, :, :, 8:16], in_=y4)
                Pm = pp.tile([P, TS, G, 5, GS], BF16, tag="Pm")
                for d in range(5):
                    nc.vector.tensor_mul(
                        out=Pm[:, :, :, d, :],
                        in0=yx[:, :, :, 0:8], in1=yx[:, :, :, d:d + 8])
                Pf = Pm.rearrange("p ts g d i -> p ts (g d i)")
                gf = gfp.tile([P, 4, CH], BF16, tag="gf")
                for m in range(4):
                    rhs5 = rp.tile([P, 5, CH], BF16, tag="rhs5")
                    for r in range(5):
                        kk = 5 * m + r
                        tr_feat(lambda ts: Pf[:, ts, kk * P:(kk + 1) * P],
                                rhs5[:, r, :])
                    pg = ps_g.tile([P, CH], F32, tag="gps")
                    for r in range(5):
                        nc.tensor.matmul(pg, geot[:, r, :], rhs5[:, r, :],
                                         start=(r == 0), stop=(r == 4))
                    nc.scalar.activation(out=gf[:, m, :], in_=pg,
                                         func=AF.Identity, bias=gbt[:, 0:1])
                # y2 = y + 0.1*geoT ; LN2 -> h
                for ts in range(TS):
                    tpg = ps_tp.tile([P, CH], BF16, tag="tp")
                    for ft in range(4):
                        nc.tensor.transpose(tpg[:, ft * P:(ft + 1) * P],
                                            gf[:, ft, ts * P:(ts + 1) * P], ident)
                    nc.vector.scalar_tensor_tensor(
                        out=y[:, ts, :], in0=tpg, scalar=0.1, in1=y[:, ts, :],
                        op0=alu.mult, op1=alu.add)
                    mv, rs = ln_stats(y[:, ts, :])
                    nc.vector.tensor_scalar(
                        out=h_sb[:, st0 + ts, :], in0=y[:, ts, :],
                        scalar1=mv[:, 0:1], scalar2=rs,
                        op0=alu.subtract, op1=alu.mult)
                    if not ln_skip:
                        nc.vector.tensor_mul(out=h_sb[:, st0 + ts, :],
                                             in0=h_sb[:, st0 + ts, :], in1=g2t)
                        nc.vector.tensor_add(out=h_sb[:, st0 + ts, :],
                                             in0=h_sb[:, st0 + ts, :], in1=b2rt)

        # ---- GeometricInteraction ----
        for c in range(NCH):
            st0 = c * TS
            xtf = xfp.tile([P, 4, CH], BF16, tag="xtf")
            for ft in range(4):
                tr_feat(lambda ts: h_sb[:, st0 + ts, ft * P:(ft + 1) * P],
                        xtf[:, ft, :])
            pv = ps_s.tile([16, CH], F32, tag="sps")
            for kt in range(4):
                nc.tensor.matmul(pv, gpv_sb[:, kt, :], xtf[:, kt, :],
                                 start=(kt == 0), stop=(kt == 3))
            pvsb = sm.tile([16, CH], BF16, tag="pvsb")
            nc.scalar.activation(out=pvsb, in_=pv, func=AF.Identity, bias=bpv_sb)
            ivT = sm.tile([G, TS, P], BF16, tag="ivT")
            for ts in range(TS):
                tp2 = ps_tp.tile([P, CH], BF16, tag="tp")
                nc.tensor.transpose(tp2[:, 0:16], pvsb[:, ts * P:(ts + 1) * P],
                                    ident[:16, :16])
                pvt = sm.tile([P, 16], BF16, tag="pvt")
                nc.vector.tensor_copy(out=pvt, in_=tp2[:, 0:16])
                iv = sm.tile([P, GS, GS], BF16, tag="iv")
                nc.vector.tensor_mul(
                    out=iv,
                    in0=pvt[:, 0:8].unsqueeze(2).to_broadcast((P, GS, GS)),
                    in1=pvt[:, 8:16].unsqueeze(1).to_broadcast((P, GS, GS)))
                tp3 = ps_tp.tile([P, CH], BF16, tag="tp")
                nc.tensor.transpose(tp3[:G, 0:P], iv.rearrange("p a b -> p (a b)"),
                                    ident)
                nc.vector.tensor_copy(out=ivT[:, ts, :], in_=tp3[:G, 0:P])
            itf = z2p.tile([P, 4, CH], BF16, tag="z2")
            for ft in range(4):
                pm = ps_mm.tile([P, CH], F32, tag="mm")
                nc.tensor.matmul(pm, giw_sb[:, ft * P:(ft + 1) * P],
                                 ivT.rearrange("p ts c -> p (ts c)"),
                                 start=True, stop=True)
                nc.scalar.activation(out=itf[:, ft, :], in_=pm, func=AF.Identity,
                                     bias=bgi_sb[:, ft:ft + 1])
            y = yp.tile([P, TS, D], F32, tag="y")
            for ts in range(TS):
                tpb = ps_tp.tile([P, CH], BF16, tag="tp")
                for ft in range(4):
                    nc.tensor.transpose(tpb[:, ft * P:(ft + 1) * P],
                                        itf[:, ft, ts * P:(ts + 1) * P], ident)
                nc.vector.tensor_add(out=y[:, ts, :], in0=tpb,
                                     in1=h_sb[:, st0 + ts, :])
                mv, rs = ln_stats(y[:, ts, :])
                nc.vector.tensor_scalar(
                    out=h_sb[:, st0 + ts, :], in0=y[:, ts, :],
                    scalar1=mv[:, 0:1], scalar2=rs,
                    op0=alu.subtract, op1=alu.mult)

        # ---- particle MLP + output ----
        for c in range(NCH):
            st0 = c * TS
            xtf = xfp.tile([P, 4, CH], BF16, tag="xtf")
            for ft in range(4):
                tr_feat(lambda ts: h_sb[:, st0 + ts, ft * P:(ft + 1) * P],
                        xtf[:, ft, :])
            z1 = z1p.tile([P, 8, CH], BF16, tag="z1")
            for mt in range(4):
                pm = ps_mm.tile([P, CH], F32, tag="mm")
                for kt in range(4):
                    nc.tensor.matmul(pm, pi1_sb[:, kt, mt * P:(mt + 1) * P],
                                     xtf[:, kt, :], start=(kt == 0), stop=(kt == 3))
                nc.scalar.activation(out=z1[:, mt, :], in_=pm, func=AF.Gelu,
                                     bias=bp1_sb[:, mt:mt + 1])
            z2 = z2p.tile([P, 4, CH], BF16, tag="z2")
            for ft in range(4):
                pm = ps_mm.tile([P, CH], F32, tag="mm")
                for kt in range(4):
                    nc.tensor.matmul(pm, pi2_sb[:, kt, ft * P:(ft + 1) * P],
                                     z1[:, kt, :], start=(kt == 0), stop=(kt == 3))
                nc.scalar.activation(out=z2[:, ft, :], in_=pm, func=AF.Identity,
                                     bias=bp2_sb[:, ft:ft + 1])
            po = ps_s.tile([16, CH], F32, tag="sps")
            for kt in range(4):
                nc.tensor.matmul(po[:4, :], ow_sb[:, kt, :], z2[:, kt, :],
                                 start=(kt == 0), stop=(kt == 3))
            xc = sm.tile([4, CH], F32, tag="xc")
            nc.sync.dma_start(out=xc, in_=xT[:, c * CH:(c + 1) * CH])
            osb = sm.tile([4, CH], F32, tag="osb")
            nc.vector.scalar_tensor_tensor(
                out=osb, in0=po[:4, :], scalar=ob_sb, in1=xc,
                op0=alu.add, op1=alu.add)
            nc.sync.dma_start(out=OUT[:, c * CH:(c + 1) * CH], in_=osb)

    nc.compile()
    return nc


def _prepack_weights(inputs):
    """Host-side weight packing. Returns (shared dict, ln_skip)."""
    f = lambda a: np.ascontiguousarray(np.asarray(a, np.float32))
    in_w, in_b = f(inputs["in_w"]), f(inputs["in_b"])
    fc1_w, fc1_b = f(inputs["fc1_w"]), f(inputs["fc1_b"])
    fc2_w, fc2_b = f(inputs["fc2_w"]), f(inputs["fc2_b"])
    geo_w, geo_b = f(inputs["geo_w"]), f(inputs["geo_b"])
    n1_g, n1_b = f(inputs["n1_g"]), f(inputs["n1_b"])
    n2_g, n2_b = f(inputs["n2_g"]), f(inputs["n2_b"])

    W1 = (n1_g[:, :, None] * fc1_w).astype(NPBF)
    b1full = fc1_b + np.einsum("ld,lde->le", n1_b, fc1_w)
    B1 = b1full.reshape(L, 8, P).transpose(0, 2, 1).copy()
    W2 = fc2_w.astype(NPBF)
    B2 = fc2_b.reshape(L, 4, P).transpose(0, 2, 1).copy()

    # modular-shift symmetric geo weights: w_mod[d,i,k], pairs (i,(i+d)%8)
    gw3 = geo_w.reshape(L, 8, 8, 8)
    wmod = np.zeros((L, 5, 8, 8), np.float32)
    ii = np.arange(8)
    for d in range(5):
        jj = (ii + d) % 8
        if d == 0:
            wmod[:, d] = gw3[:, ii, ii, :]
        elif d == 4:
            wmod[:, d] = 0.5 * (gw3[:, ii, jj, :] + gw3[:, jj, ii, :])
        else:
            wmod[:, d] = gw3[:, ii, jj, :] + gw3[:, jj, ii, :]
    # block matrix for one 128-col output block (16 groups); chunks repeat
    # with period 5 across the 20 feature chunks.
    blk = np.zeros((L, 16, 5, 8, 16, 8), np.float32)
    for g in range(16):
        blk[:, g, :, :, g, :] = wmod
    GEOS = blk.reshape(L, 640, 128).reshape(L, 5, 128, 128).astype(NPBF)
    GBT = np.tile(geo_b, (1, 16)).reshape(L, P, 1).astype(np.float32)

    BIN = in_b.reshape(4, P).T.copy()
    GPV = np.concatenate(
        [f(inputs["gi_pos_w"]), f(inputs["gi_vel_w"])], axis=1
    ).reshape(4, P, 16).astype(NPBF)
    BPV = np.concatenate([f(inputs["gi_pos_b"]), f(inputs["gi_vel_b"])])[:, None]
    GIW = f(inputs["gi_int_w"]).astype(NPBF)
    BGI = f(inputs["gi_int_b"]).reshape(4, P).T.copy()
    gn_g, gn_b = f(inputs["gi_n_g"]), f(inputs["gi_n_b"])
    PI1 = (gn_g[:, None] * f(inputs["pi1_w"])).astype(NPBF)
    bp1full = f(inputs["pi1_b"]) + gn_b @ f(inputs["pi1_w"])
    BP1 = bp1full.reshape(4, P).T.copy()
    PI2 = f(inputs["pi2_w"]).astype(NPBF)
    BP2 = f(inputs["pi2_b"]).reshape(4, P).T.copy()
    OW = f(inputs["out_w"]).reshape(4, P, 4).astype(NPBF)
    OB = f(inputs["out_b"])[:, None]

    ln_skip = (np.all(n1_g == 1.0) and np.all(n1_b == 0.0)
               and np.all(n2_g == 1.0) and np.all(n2_b == 0.0))
    shared = dict(W1=W1, B1=B1, W2=W2, B2=B2, GEOS=GEOS, GBT=GBT,
                  WIN=in_w.astype(NPBF), BIN=BIN, GPV=GPV, BPV=BPV,
                  GIW=GIW, BGI=BGI, PI1=PI1, BP1=BP1, PI2=PI2, BP2=BP2,
                  OW=OW, OB=OB)
    if not ln_skip:
        shared["G2R"] = np.ascontiguousarray(
            np.broadcast_to(n2_g[:, None, :], (L, P, D)), np.float32)
        shared["B2R"] = np.ascontiguousarray(
            np.broadcast_to(n2_b[:, None, :], (L, P, D)), np.float32)
    shared = {k: np.ascontiguousarray(v) for k, v in shared.items()}
    return shared, ln_skip


_NC_CACHE = {}


def _get_compiled(T, CH, ln_skip):
    key = (T, CH, ln_skip)
    if key not in _NC_CACHE:
        _NC_CACHE[key] = build_nc(T, CH, ln_skip)
    return _NC_CACHE[key]


class _FastRunner:
    """Caches the jitted shard_map executable and device-resident weights.

    Weight inputs are validated by byte-comparison against the cached host
    copies on every call; only x-derived tensors are shipped per call.
    """

    def __init__(self, nc, n_cores):
        import jax
        from jax.sharding import Mesh, PartitionSpec, NamedSharding
        from jax.experimental.shard_map import shard_map
        from concourse.bass2jax import (_bass_exec_p, install_neuronx_cc_hook,
                                        partition_id_tensor)
        install_neuronx_cc_hook()
        self.jax = jax
        self.nc = nc
        self.n_cores = n_cores
        partition_name = (nc.partition_id_tensor.name
                          if nc.partition_id_tensor else None)
        in_names, out_names, out_avals, zero_outs = [], [], [], []
        for alloc in nc.m.functions[0].allocations:
            if not isinstance(alloc, mybir.MemoryLocationSet):
                continue
            name = alloc.memorylocations[0].name
            if alloc.kind == "ExternalInput":
                if name != partition_name:
                    in_names.append(name)
            elif alloc.kind == "ExternalOutput":
                out_names.append(name)
                shape = tuple(alloc.tensor_shape)
                dtype = mybir.dt.np(alloc.dtype)
                out_avals.append(jax.core.ShapedArray(shape, dtype))
                zero_outs.append(np.zeros(shape, dtype))
        self.in_names = in_names
        self.out_names = out_names
        self.out_avals = out_avals
        self.zero_outs = zero_outs
        n_params = len(in_names)
        n_outs = len(out_avals)
        in_names_full = in_names + out_names + (
            [partition_name] if partition_name else [])

        def _body(*args):
            operands = list(args)
            if partition_name is not None:
                operands.append(partition_id_tensor())
            outs = _bass_exec_p.bind(
                *operands, out_avals=tuple(out_avals),
                in_names=tuple(in_names_full), out_names=tuple(out_names),
                lowering_input_output_aliases=(), sim_require_finite=True,
                sim_require_nnan=True, nc=nc)
            return tuple(outs)

        devices = jax.devices()[:n_cores]
        self.mesh = Mesh(np.asarray(devices), ("core",))
        self.sharding = NamedSharding(self.mesh, PartitionSpec("core"))
        in_specs = (PartitionSpec("core"),) * (n_params + n_outs)
        out_specs = (PartitionSpec("core"),) * len(out_names)
        donate = tuple(range(n_params, n_params + n_outs))
        self.sharded = jax.jit(
            shard_map(_body, mesh=self.mesh, in_specs=in_specs,
                      out_specs=out_specs, check_rep=False),
            donate_argnums=donate, keep_unused=True)
        self._host_cache = {}   # name -> host np array (concat)
        self._dev_cache = {}    # name -> device array

    def run(self, per_core_maps, volatile):
        """per_core_maps: list of dicts; volatile: set of names shipped fresh."""
        jax = self.jax
        n = self.n_cores
        args = []
        for name in self.in_names:
            cat = np.concatenate([np.asarray(per_core_maps[c][name])
                                  for c in range(n)], axis=0)
            if name in volatile:
                args.append(cat)
                continue
            cached = self._host_cache.get(name)
            if (cached is not None and cached.shape == cat.shape
                    and cached.dtype == cat.dtype
                    and np.array_equal(cached, cat)):
                args.append(self._dev_cache[name])
            else:
                dev = jax.device_put(cat, self.sharding)
                self._host_cache[name] = cat
                self._dev_cache[name] = dev
                args.append(dev)
        for z in self.zero_outs:
            args.append(np.zeros((n * z.shape[0], *z.shape[1:]), z.dtype))
        out_arrs = self.sharded(*args)
        outs = []
        for i, name in enumerate(self.out_names):
            a = np.asarray(out_arrs[i]).reshape(n, *self.out_avals[i].shape)
            outs.append(a)
        return {name: outs[i] for i, name in enumerate(self.out_names)}


_RUNNER_CACHE = {}


def _get_runner(nc):
    key = id(nc)
    if key not in _RUNNER_CACHE:
        _RUNNER_CACHE[key] = _FastRunner(nc, NCORES)
    return _RUNNER_CACHE[key]


def kernel(**inputs):
    x = np.asarray(inputs["x"], np.float32)
    B, N, _ = x.shape
    T = B * N // NCORES
    shared, ln_skip = _prepack_weights(inputs)
    nc = _get_compiled(T, 512, ln_skip)

    xs = np.ascontiguousarray(x.reshape(-1, 4))
    in_maps = []
    for c in range(NCORES):
        m = dict(shared)
        xTc = np.ascontiguousarray(xs[c * T:(c + 1) * T].T)
        m["xT"] = xTc
        m["XB"] = xTc.astype(NPBF)
        in_maps.append(m)

    try:
        runner = _get_runner(nc)
        res = runner.run(in_maps, volatile={"xT", "XB"})
        outs = [res["OUT"][c].T for c in range(NCORES)]
    except Exception as e:  # pragma: no cover - safety net
        import traceback
        traceback.print_exc()
        print(f"fast path failed ({e!r}); falling back to run_bass_kernel_spmd")
        res = run_bass_kernel_spmd(nc, in_maps, core_ids=list(range(NCORES)))
        outs = [res.results[c]["OUT"].T for c in range(NCORES)]
    full = np.concatenate(outs, axis=0).reshape(B, N, 4).astype(np.float32)
    return full


# revision 8
# speedup vs baseline: 114.5666x; 3.5373x over previous
"""Trainium2 Bass kernel for nn_HCNetFull (dense_mlp), 8-core data parallel.

Strategy: shard the 32768 tokens across 8 NeuronCores (4096 each).
Token-major bf16 activations resident in SBUF; all matmuls/transposes in
bf16 (PE 1 cyc/col vs 4 for fp32); fp32 accumulation in PSUM and fp32
LayerNorm statistics.  The per-group outer-product mixing uses the
modular-shift symmetric factorization: x_i*x_j terms are covered by the
40 products x_i * x_{(i+d)%8}, d=0..4, so the contraction is a 2560->512
block matmul whose 128x128 lhsT chunks repeat with period 5.
LN1 of layers >=1 is skipped: its input is the previous LN2 output
(already zero-mean/unit-var, and n1_g=1, n1_b=0), so LN1 is an identity
up to O(eps)=1e-5.

Host side: the jitted shard_map executable and the device-resident
weight arrays are cached across kernel() calls (the axon tunnel moves
~40MB/s, so re-shipping 130MB of replicated weights per call dominates
wall time otherwise).  Weights are revalidated by byte comparison
against the cached host copies each call.
"""

import numpy as np
from contextlib import ExitStack

import concourse.bass as bass
import concourse.tile as tile
from concourse import bacc, mybir
from concourse.bass_utils import run_bass_kernel_spmd
from concourse.masks import make_identity

F32 = mybir.dt.float32
BF16 = mybir.dt.bfloat16
NPBF = mybir.dt.np(BF16)
D, DD, L, GS, G, P = 512, 1024, 8, 8, 64, 128
NCORES = 8
AF = mybir.ActivationFunctionType
ALU = None


def _alu():
    global ALU
    if ALU is None:
        ALU = mybir.AluOpType
    return ALU


def build_nc(T, CH, ln_skip):
    alu = _alu()
    NT = T // P
    NCH = T // CH
    TS = CH // P

    nc = bacc.Bacc("TRN2", target_bir_lowering=False, debug=False)

    def din(name, shape, dt=BF16):
        return nc.dram_tensor(name, list(shape), dt, kind="ExternalInput")

    xT = din("xT", (4, T), F32)
    XB = din("XB", (4, T))
    W1 = din("W1", (L, D, DD)); B1 = din("B1", (L, P, 8), F32)
    W2 = din("W2", (L, DD, D)); B2 = din("B2", (L, P, 4), F32)
    GEOS = din("GEOS", (L, 5, P, P)); GBT = din("GBT", (L, P, 1), F32)
    WIN = din("WIN", (4, D)); BIN = din("BIN", (P, 4), F32)
    GPV = din("GPV", (4, P, 16)); BPV = din("BPV", (16, 1), F32)
    GIW = din("GIW", (G, D)); BGI = din("BGI", (P, 4), F32)
    PI1 = din("PI1", (D, D)); BP1 = din("BP1", (P, 4), F32)
    PI2 = din("PI2", (D, D)); BP2 = din("BP2", (P, 4), F32)
    OW = din("OW", (4, P, 4)); OB = din("OB", (4, 1), F32)
    if not ln_skip:
        G2R = din("G2R", (L, P, D), F32); B2R = din("B2R", (L, P, D), F32)
    OUT = nc.dram_tensor("OUT", [4, T], F32, kind="ExternalOutput")

    with tile.TileContext(nc) as tc, ExitStack() as _px:
        cst = _px.enter_context(tc.tile_pool(name="cst", bufs=1))
        wl = _px.enter_context(tc.tile_pool(name="wl", bufs=2))
        hp = _px.enter_context(tc.tile_pool(name="hp", bufs=1))
        xfp = _px.enter_context(tc.tile_pool(name="xfp", bufs=2))
        z1p = _px.enter_context(tc.tile_pool(name="z1p", bufs=2))
        z2p = _px.enter_context(tc.tile_pool(name="z2p", bufs=2))
        yp = _px.enter_context(tc.tile_pool(name="yp", bufs=2))
        yep = _px.enter_context(tc.tile_pool(name="yep", bufs=1))
        pp = _px.enter_context(tc.tile_pool(name="pp", bufs=1))
        rp = _px.enter_context(tc.tile_pool(name="rp", bufs=2))
        gfp = _px.enter_context(tc.tile_pool(name="gfp", bufs=2))
        sm = _px.enter_context(tc.tile_pool(name="sm", bufs=2))
        st = _px.enter_context(tc.tile_pool(name="st", bufs=8))
        ps_tp = _px.enter_context(tc.tile_pool(name="ps_tp", bufs=2, space="PSUM"))
        ps_mm = _px.enter_context(tc.tile_pool(name="ps_mm", bufs=2, space="PSUM"))
        ps_g = _px.enter_context(tc.tile_pool(name="ps_g", bufs=2, space="PSUM"))
        ps_s = _px.enter_context(tc.tile_pool(name="ps_s", bufs=1, space="PSUM"))

        ident = cst.tile([P, P], BF16)
        make_identity(nc, ident)
        eps_t = cst.tile([P, 1], F32)
        nc.vector.memset(eps_t, 1e-5)
        win_sb = cst.tile([4, 4, P], BF16)
        nc.sync.dma_start(out=win_sb, in_=WIN[:, :].rearrange("p (mt c) -> p mt c", c=P))
        bin_sb = cst.tile([P, 4], F32)
        nc.sync.dma_start(out=bin_sb, in_=BIN[:, :])
        gpv_sb = cst.tile([P, 4, 16], BF16)
        nc.sync.dma_start(out=gpv_sb, in_=GPV[:, :, :].rearrange("kt p c -> p kt c"))
        bpv_sb = cst.tile([16, 1], F32)
        nc.sync.dma_start(out=bpv_sb, in_=BPV[:, :])
        giw_sb = cst.tile([G, D], BF16)
        nc.sync.dma_start(out=giw_sb, in_=GIW[:, :])
        bgi_sb = cst.tile([P, 4], F32)
        nc.sync.dma_start(out=bgi_sb, in_=BGI[:, :])
        pi1_sb = cst.tile([P, 4, D], BF16)
        nc.sync.dma_start(out=pi1_sb, in_=PI1[:, :].rearrange("(kt p) c -> p kt c", p=P))
        pi2_sb = cst.tile([P, 4, D], BF16)
        nc.sync.dma_start(out=pi2_sb, in_=PI2[:, :].rearrange("(kt p) c -> p kt c", p=P))
        bp1_sb = cst.tile([P, 4], F32)
        nc.sync.dma_start(out=bp1_sb, in_=BP1[:, :])
        bp2_sb = cst.tile([P, 4], F32)
        nc.sync.dma_start(out=bp2_sb, in_=BP2[:, :])
        ow_sb = cst.tile([P, 4, 4], BF16)
        nc.sync.dma_start(out=ow_sb, in_=OW[:, :, :].rearrange("kt p c -> p kt c"))
        ob_sb = cst.tile([4, 1], F32)
        nc.sync.dma_start(out=ob_sb, in_=OB[:, :])

        h_sb = hp.tile([P, NT, D], BF16)

        def ln_stats(src):
            s6 = st.tile([P, 6], F32, tag="s6")
            nc.vector.bn_stats(out=s6, in_=src)
            mv = st.tile([P, 2], F32, tag="mv")
            nc.vector.bn_aggr(out=mv, in_=s6)
            sd = st.tile([P, 1], F32, tag="sd")
            nc.scalar.activation(out=sd, in_=mv[:, 1:2], func=AF.Sqrt, bias=eps_t)
            rs = st.tile([P, 1], F32, tag="rs")
            nc.vector.reciprocal(out=rs, in_=sd)
            return mv, rs

        # rotate PSUM->SBUF copies (GPSIMD cannot access PSUM)
        cp_engines = [nc.vector, nc.scalar]
        cp_i = [0]

        def cp(out, in_):
            e = cp_engines[cp_i[0] % len(cp_engines)]
            cp_i[0] += 1
            if e is nc.scalar:
                e.copy(out=out, in_=in_)
            else:
                e.tensor_copy(out=out, in_=in_)

        def tr_feat(src_ts, dst):
            """src_ts(ts)->AP [128 tok,128 f]; dst [128 f, CH tok] sbuf."""
            tpb = ps_tp.tile([P, CH], BF16, tag="tp")
            for ts in range(TS):
                nc.tensor.transpose(tpb[:, ts * P:(ts + 1) * P], src_ts(ts), ident)
            cp(dst, tpb)

        # ---- input projection: h0 = x @ Win + bin (token-major bf16) ----
        for c in range(NCH):
            xcb = sm.tile([4, CH], BF16, tag="xcb")
            nc.sync.dma_start(out=xcb, in_=XB[:, c * CH:(c + 1) * CH])
            h0f = sm.tile([P, 4, CH], BF16, tag="h0f")
            for mt in range(4):
                pm = ps_mm.tile([P, CH], F32, tag="mm")
                nc.tensor.matmul(pm, win_sb[:, mt, :], xcb, start=True, stop=True)
                nc.scalar.activation(out=h0f[:, mt, :], in_=pm, func=AF.Identity,
                                     bias=bin_sb[:, mt:mt + 1])
            for ts in range(TS):
                tpb = ps_tp.tile([P, CH], BF16, tag="tp")
                for ft in range(4):
                    nc.tensor.transpose(tpb[:, ft * P:(ft + 1) * P],
                                        h0f[:, ft, ts * P:(ts + 1) * P], ident)
                cp(h_sb[:, c * TS + ts, :], tpb)

        # ---- transformer layers ----
        for l in range(L):
            w1t = wl.tile([P, 4, DD], BF16, tag="w1")
            nc.sync.dma_start(out=w1t, in_=W1[l].rearrange("(kt p) c -> p kt c", p=P))
            w2t = wl.tile([P, 8, D], BF16, tag="w2")
            nc.sync.dma_start(out=w2t, in_=W2[l].rearrange("(kt p) c -> p kt c", p=P))
            geot = wl.tile([P, 5, P], BF16, tag="geo")
            nc.sync.dma_start(out=geot, in_=GEOS[l].rearrange("r p c -> p r c"))
            b1t = wl.tile([P, 8], F32, tag="b1")
            nc.sync.dma_start(out=b1t, in_=B1[l])
            b2t = wl.tile([P, 4], F32, tag="b2")
            nc.sync.dma_start(out=b2t, in_=B2[l])
            gbt = wl.tile([P, 1], F32, tag="gb")
            nc.sync.dma_start(out=gbt, in_=GBT[l])
            if not ln_skip:
                g2t = wl.tile([P, D], F32, tag="g2")
                nc.sync.dma_start(out=g2t, in_=G2R[l])
                b2rt = wl.tile([P, D], F32, tag="b2r")
                nc.sync.dma_start(out=b2rt, in_=B2R[l])

            for c in range(NCH):
                st0 = c * TS
                # LN1: identity for l>=1 in the ln_skip regime
                if l == 0 or not ln_skip:
                    xln = xfp.tile([P, TS, D], BF16, tag="xln")
                    for ts in range(TS):
                        mv, rs = ln_stats(h_sb[:, st0 + ts, :])
                        nc.vector.tensor_scalar(
                            out=xln[:, ts, :], in0=h_sb[:, st0 + ts, :],
                            scalar1=mv[:, 0:1], scalar2=rs,
                            op0=alu.subtract, op1=alu.mult)
                    src = lambda ts, f0: xln[:, ts, f0:f0 + P]
                else:
                    src = lambda ts, f0: h_sb[:, st0 + ts, f0:f0 + P]
                xtf = xfp.tile([P, 4, CH], BF16, tag="xtf")
                for ft in range(4):
                    tr_feat(lambda ts: src(ts, ft * P), xtf[:, ft, :])
                # fc1 + gelu
                z1 = z1p.tile([P, 8, CH], BF16, tag="z1")
                for mt in range(8):
                    pm = ps_mm.tile([P, CH], F32, tag="mm")
                    for kt in range(4):
                        nc.tensor.matmul(pm, w1t[:, kt, mt * P:(mt + 1) * P],
                                         xtf[:, kt, :], start=(kt == 0), stop=(kt == 3))
                    nc.scalar.activation(out=z1[:, mt, :], in_=pm, func=AF.Gelu,
                                         bias=b1t[:, mt:mt + 1])
                # fc2
                z2 = z2p.tile([P, 4, CH], BF16, tag="z2")
                for ft in range(4):
                    pm = ps_mm.tile([P, CH], F32, tag="mm")
                    for kt in range(8):
                        nc.tensor.matmul(pm, w2t[:, kt, ft * P:(ft + 1) * P],
                                         z1[:, kt, :], start=(kt == 0), stop=(kt == 7))
                    nc.scalar.activation(out=z2[:, ft, :], in_=pm, func=AF.Identity,
                                         bias=b2t[:, ft:ft + 1])
                # transpose back + residual (fp32 y)
                y = yp.tile([P, TS, D], F32, tag="y")
                for ts in range(TS):
                    tpb = ps_tp.tile([P, CH], BF16, tag="tp")
                    for ft in range(4):
                        nc.tensor.transpose(tpb[:, ft * P:(ft + 1) * P],
                                            z2[:, ft, ts * P:(ts + 1) * P], ident)
                    nc.vector.tensor_add(out=y[:, ts, :], in0=tpb,
                                         in1=h_sb[:, st0 + ts, :])
                # geometric mixing: P features f = 40g + 8d + i,
                # P[g,d,i] = y[g,i] * y[g,(i+d)%8]
                yx = yep.tile([P, TS, G, 16], BF16, tag="yx")
                y4 = y.rearrange("p ts (g i) -> p ts g i", i=GS)
                nc.gpsimd.tensor_copy(out=yx[:, :, :, 0:8], in_=y4)
                nc.gpsimd.tensor_copy(out=yx[:, :, :, 8:16], in_=y4)
                Pm = pp.tile([P, TS, G, 5, GS], BF16, tag="Pm")
                for d in range(5):
                    nc.vector.tensor_mul(
                        out=Pm[:, :, :, d, :],
                        in0=yx[:, :, :, 0:8], in1=yx[:, :, :, d:d + 8])
                Pf = Pm.rearrange("p ts g d i -> p ts (g d i)")
                gf = gfp.tile([P, 4, CH], BF16, tag="gf")
                for m in range(4):
                    rhs5 = rp.tile([P, 5, CH], BF16, tag="rhs5")
                    for r in range(5):
                        kk = 5 * m + r
                        tr_feat(lambda ts: Pf[:, ts, kk * P:(kk + 1) * P],
                                rhs5[:, r, :])
                    pg = ps_g.tile([P, CH], F32, tag="gps")
                    for r in range(5):
                        nc.tensor.matmul(pg, geot[:, r, :], rhs5[:, r, :],
                                         start=(r == 0), stop=(r == 4))
                    nc.scalar.activation(out=gf[:, m, :], in_=pg,
                                         func=AF.Identity, bias=gbt[:, 0:1])
                # y2 = y + 0.1*geoT ; LN2 -> h
                for ts in range(TS):
                    tpg = ps_tp.tile([P, CH], BF16, tag="tp")
                    for ft in range(4):
                        nc.tensor.transpose(tpg[:, ft * P:(ft + 1) * P],
                                            gf[:, ft, ts * P:(ts + 1) * P], ident)
                    nc.vector.scalar_tensor_tensor(
                        out=y[:, ts, :], in0=tpg, scalar=0.1, in1=y[:, ts, :],
                        op0=alu.mult, op1=alu.add)
                    mv, rs = ln_stats(y[:, ts, :])
                    nc.vector.tensor_scalar(
                        out=h_sb[:, st0 + ts, :], in0=y[:, ts, :],
                        scalar1=mv[:, 0:1], scalar2=rs,
                        op0=alu.subtract, op1=alu.mult)
                    if not ln_skip:
                        nc.vector.tensor_mul(out=h_sb[:, st0 + ts, :],
                                             in0=h_sb[:, st0 + ts, :], in1=g2t)
                        nc.vector.tensor_add(out=h_sb[:, st0 + ts, :],
                                             in0=h_sb[:, st0 + ts, :], in1=b2rt)

        # ---- GeometricInteraction ----
        for c in range(NCH):
            st0 = c * TS
            xtf = xfp.tile([P, 4, CH], BF16, tag="xtf")
            for ft in range(4):
                tr_feat(lambda ts: h_sb[:, st0 + ts, ft * P:(ft + 1) * P],
                        xtf[:, ft, :])
            pv = ps_s.tile([16, CH], F32, tag="sps")
            for kt in range(4):
                nc.tensor.matmul(pv, gpv_sb[:, kt, :], xtf[:, kt, :],
                                 start=(kt == 0), stop=(kt == 3))
            pvsb = sm.tile([16, CH], BF16, tag="pvsb")
            nc.scalar.activation(out=pvsb, in_=pv, func=AF.Identity, bias=bpv_sb)
            ivT = sm.tile([G, TS, P], BF16, tag="ivT")
            for ts in range(TS):
                tp2 = ps_tp.tile([P, CH], BF16, tag="tp")
                nc.tensor.transpose(tp2[:, 0:16], pvsb[:, ts * P:(ts + 1) * P],
                                    ident[:16, :16])
                pvt = sm.tile([P, 16], BF16, tag="pvt")
                nc.vector.tensor_copy(out=pvt, in_=tp2[:, 0:16])
                iv = sm.tile([P, GS, GS], BF16, tag="iv")
                nc.vector.tensor_mul(
                    out=iv,
                    in0=pvt[:, 0:8].unsqueeze(2).to_broadcast((P, GS, GS)),
                    in1=pvt[:, 8:16].unsqueeze(1).to_broadcast((P, GS, GS)))
                tp3 = ps_tp.tile([P, CH], BF16, tag="tp")
                nc.tensor.transpose(tp3[:G, 0:P], iv.rearrange("p a b -> p (a b)"),
                                    ident)
                nc.vector.tensor_copy(out=ivT[:, ts, :], in_=tp3[:G, 0:P])
            itf = z2p.tile([P, 4, CH], BF16, tag="z2")
            for ft in range(4):
                pm = ps_mm.tile([P, CH], F32, tag="mm")
                nc.tensor.matmul(pm, giw_sb[:, ft * P:(ft + 1) * P],
                                 ivT.rearrange("p ts c -> p (ts c)"),
                                 start=True, stop=True)
                nc.scalar.activation(out=itf[:, ft, :], in_=pm, func=AF.Identity,
                                     bias=bgi_sb[:, ft:ft + 1])
            y = yp.tile([P, TS, D], F32, tag="y")
            for ts in range(TS):
                tpb = ps_tp.tile([P, CH], BF16, tag="tp")
                for ft in range(4):
                    nc.tensor.transpose(tpb[:, ft * P:(ft + 1) * P],
                                        itf[:, ft, ts * P:(ts + 1) * P], ident)
                nc.vector.tensor_add(out=y[:, ts, :], in0=tpb,
                                     in1=h_sb[:, st0 + ts, :])
                mv, rs = ln_stats(y[:, ts, :])
                nc.vector.tensor_scalar(
                    out=h_sb[:, st0 + ts, :], in0=y[:, ts, :],
                    scalar1=mv[:, 0:1], scalar2=rs,
                    op0=alu.subtract, op1=alu.mult)

        # ---- particle MLP + output ----
        for c in range(NCH):
            st0 = c * TS
            xtf = xfp.tile([P, 4, CH], BF16, tag="xtf")
            for ft in range(4):
                tr_feat(lambda ts: h_sb[:, st0 + ts, ft * P:(ft + 1) * P],
                        xtf[:, ft, :])
            z1 = z1p.tile([P, 8, CH], BF16, tag="z1")
            for mt in range(4):
                pm = ps_mm.tile([P, CH], F32, tag="mm")
                for kt in range(4):
                    nc.tensor.matmul(pm, pi1_sb[:, kt, mt * P:(mt + 1) * P],
                                     xtf[:, kt, :], start=(kt == 0), stop=(kt == 3))
                nc.scalar.activation(out=z1[:, mt, :], in_=pm, func=AF.Gelu,
                                     bias=bp1_sb[:, mt:mt + 1])
            z2 = z2p.tile([P, 4, CH], BF16, tag="z2")
            for ft in range(4):
                pm = ps_mm.tile([P, CH], F32, tag="mm")
                for kt in range(4):
                    nc.tensor.matmul(pm, pi2_sb[:, kt, ft * P:(ft + 1) * P],
                                     z1[:, kt, :], start=(kt == 0), stop=(kt == 3))
                nc.scalar.activation(out=z2[:, ft, :], in_=pm, func=AF.Identity,
                                     bias=bp2_sb[:, ft:ft + 1])
            po = ps_s.tile([16, CH], F32, tag="sps")
            for kt in range(4):
                nc.tensor.matmul(po[:4, :], ow_sb[:, kt, :], z2[:, kt, :],
                                 start=(kt == 0), stop=(kt == 3))
            xc = sm.tile([4, CH], F32, tag="xc")
            nc.sync.dma_start(out=xc, in_=xT[:, c * CH:(c + 1) * CH])
            osb = sm.tile([4, CH], F32, tag="osb")
            nc.vector.scalar_tensor_tensor(
                out=osb, in0=po[:4, :], scalar=ob_sb, in1=xc,
                op0=alu.add, op1=alu.add)
            nc.sync.dma_start(out=OUT[:, c * CH:(c + 1) * CH], in_=osb)

    nc.compile()
    return nc


def _prepack_weights(inputs):
    """Host-side weight packing. Returns (shared dict, ln_skip)."""
    f = lambda a: np.ascontiguousarray(np.asarray(a, np.float32))
    in_w, in_b = f(inputs["in_w"]), f(inputs["in_b"])
    fc1_w, fc1_b = f(inputs["fc1_w"]), f(inputs["fc1_b"])
    fc2_w, fc2_b = f(inputs["fc2_w"]), f(inputs["fc2_b"])
    geo_w, geo_b = f(inputs["geo_w"]), f(inputs["geo_b"])
    n1_g, n1_b = f(inputs["n1_g"]), f(inputs["n1_b"])
    n2_g, n2_b = f(inputs["n2_g"]), f(inputs["n2_b"])

    W1 = (n1_g[:, :, None] * fc1_w).astype(NPBF)
    b1full = fc1_b + np.einsum("ld,lde->le", n1_b, fc1_w)
    B1 = b1full.reshape(L, 8, P).transpose(0, 2, 1).copy()
    W2 = fc2_w.astype(NPBF)
    B2 = fc2_b.reshape(L, 4, P).transpose(0, 2, 1).copy()

    # modular-shift symmetric geo weights: w_mod[d,i,k], pairs (i,(i+d)%8)
    gw3 = geo_w.reshape(L, 8, 8, 8)
    wmod = np.zeros((L, 5, 8, 8), np.float32)
    ii = np.arange(8)
    for d in range(5):
        jj = (ii + d) % 8
        if d == 0:
            wmod[:, d] = gw3[:, ii, ii, :]
        elif d == 4:
            wmod[:, d] = 0.5 * (gw3[:, ii, jj, :] + gw3[:, jj, ii, :])
        else:
            wmod[:, d] = gw3[:, ii, jj, :] + gw3[:, jj, ii, :]
    # block matrix for one 128-col output block (16 groups); chunks repeat
    # with period 5 across the 20 feature chunks.
    blk = np.zeros((L, 16, 5, 8, 16, 8), np.float32)
    for g in range(16):
        blk[:, g, :, :, g, :] = wmod
    GEOS = blk.reshape(L, 640, 128).reshape(L, 5, 128, 128).astype(NPBF)
    GBT = np.tile(geo_b, (1, 16)).reshape(L, P, 1).astype(np.float32)

    BIN = in_b.reshape(4, P).T.copy()
    GPV = np.concatenate(
        [f(inputs["gi_pos_w"]), f(inputs["gi_vel_w"])], axis=1
    ).reshape(4, P, 16).astype(NPBF)
    BPV = np.concatenate([f(inputs["gi_pos_b"]), f(inputs["gi_vel_b"])])[:, None]
    GIW = f(inputs["gi_int_w"]).astype(NPBF)
    BGI = f(inputs["gi_int_b"]).reshape(4, P).T.copy()
    gn_g, gn_b = f(inputs["gi_n_g"]), f(inputs["gi_n_b"])
    PI1 = (gn_g[:, None] * f(inputs["pi1_w"])).astype(NPBF)
    bp1full = f(inputs["pi1_b"]) + gn_b @ f(inputs["pi1_w"])
    BP1 = bp1full.reshape(4, P).T.copy()
    PI2 = f(inputs["pi2_w"]).astype(NPBF)
    BP2 = f(inputs["pi2_b"]).reshape(4, P).T.copy()
    OW = f(inputs["out_w"]).reshape(4, P, 4).astype(NPBF)
    OB = f(inputs["out_b"])[:, None]

    ln_skip = (np.all(n1_g == 1.0) and np.all(n1_b == 0.0)
               and np.all(n2_g == 1.0) and np.all(n2_b == 0.0))
    shared = dict(W1=W1, B1=B1, W2=W2, B2=B2, GEOS=GEOS, GBT=GBT,
                  WIN=in_w.astype(NPBF), BIN=BIN, GPV=GPV, BPV=BPV,
                  GIW=GIW, BGI=BGI, PI1=PI1, BP1=BP1, PI2=PI2, BP2=BP2,
                  OW=OW, OB=OB)
    if not ln_skip:
        shared["G2R"] = np.ascontiguousarray(
            np.broadcast_to(n2_g[:, None, :], (L, P, D)), np.float32)
        shared["B2R"] = np.ascontiguousarray(
            np.broadcast_to(n2_b[:, None, :], (L, P, D)), np.float32)
    shared = {k: np.ascontiguousarray(v) for k, v in shared.items()}
    return shared, ln_skip


_NC_CACHE = {}


def _get_compiled(T, CH, ln_skip):
    key = (T, CH, ln_skip)
    if key not in _NC_CACHE:
        _NC_CACHE[key] = build_nc(T, CH, ln_skip)
    return _NC_CACHE[key]


class _FastRunner:
    """Caches the jitted shard_map executable and device-resident weights.

    Weight inputs are validated by byte-comparison against the cached host
    copies on every call; only x-derived tensors are shipped per call.
    """

    def __init__(self, nc, n_cores):
        import jax
        from jax.sharding import Mesh, PartitionSpec, NamedSharding
        from jax.experimental.shard_map import shard_map
        from concourse.bass2jax import (_bass_exec_p, install_neuronx_cc_hook,
                                        partition_id_tensor)
        install_neuronx_cc_hook()
        self.jax = jax
        self.nc = nc
        self.n_cores = n_cores
        partition_name = (nc.partition_id_tensor.name
                          if nc.partition_id_tensor else None)
        in_names, out_names, out_avals, zero_outs = [], [], [], []
        for alloc in nc.m.functions[0].allocations:
            if not isinstance(alloc, mybir.MemoryLocationSet):
                continue
            name = alloc.memorylocations[0].name
            if alloc.kind == "ExternalInput":
                if name != partition_name:
                    in_names.append(name)
            elif alloc.kind == "ExternalOutput":
                out_names.append(name)
                shape = tuple(alloc.tensor_shape)
                dtype = mybir.dt.np(alloc.dtype)
                out_avals.append(jax.core.ShapedArray(shape, dtype))
                zero_outs.append(np.zeros(shape, dtype))
        self.in_names = in_names
        self.out_names = out_names
        self.out_avals = out_avals
        self.zero_outs = zero_outs
        n_params = len(in_names)
        n_outs = len(out_avals)
        in_names_full = in_names + out_names + (
            [partition_name] if partition_name else [])

        def _body(*args):
            operands = list(args)
            if partition_name is not None:
                operands.append(partition_id_tensor())
            outs = _bass_exec_p.bind(
                *operands, out_avals=tuple(out_avals),
                in_names=tuple(in_names_full), out_names=tuple(out_names),
                lowering_input_output_aliases=(), sim_require_finite=True,
                sim_require_nnan=True, nc=nc)
            return tuple(outs)

        devices = jax.devices()[:n_cores]
        self.mesh = Mesh(np.asarray(devices), ("core",))
        self.sharding = NamedSharding(self.mesh, PartitionSpec("core"))
        in_specs = (PartitionSpec("core"),) * (n_params + n_outs)
        out_specs = (PartitionSpec("core"),) * len(out_names)
        donate = tuple(range(n_params, n_params + n_outs))
        self.sharded = jax.jit(
            shard_map(_body, mesh=self.mesh, in_specs=in_specs,
                      out_specs=out_specs, check_rep=False),
            donate_argnums=donate, keep_unused=True)
        self._host_cache = {}   # name -> host np array (concat)
        self._dev_cache = {}    # name -> device array

    def run(self, shared, volatile_maps):
        """shared: dict name->array (replicated); volatile_maps: name->list of
        per-core arrays, shipped fresh every call."""
        jax = self.jax
        n = self.n_cores
        args = []
        for name in self.in_names:
            if name in volatile_maps:
                args.append(np.concatenate(
                    [np.asarray(a) for a in volatile_maps[name]], axis=0))
                continue
            arr = shared[name]
            cached = self._host_cache.get(name)
            if cached is not None and cached is arr:
                args.append(self._dev_cache[name])
                continue
            if (cached is not None and cached.shape == arr.shape
                    and cached.dtype == arr.dtype
                    and np.array_equal(cached, arr)):
                self._host_cache[name] = arr
                args.append(self._dev_cache[name])
                continue
            cat = np.concatenate([arr] * n, axis=0)
            dev = jax.device_put(cat, self.sharding)
            self._host_cache[name] = arr
            self._dev_cache[name] = dev
            args.append(dev)
        for z in self.zero_outs:
            args.append(np.zeros((n * z.shape[0], *z.shape[1:]), z.dtype))
        out_arrs = self.sharded(*args)
        outs = []
        for i, name in enumerate(self.out_names):
            a = np.asarray(out_arrs[i]).reshape(n, *self.out_avals[i].shape)
            outs.append(a)
        return {name: outs[i] for i, name in enumerate(self.out_names)}


_RUNNER_CACHE = {}


def _get_runner(nc):
    key = id(nc)
    if key not in _RUNNER_CACHE:
        _RUNNER_CACHE[key] = _FastRunner(nc, NCORES)
    return _RUNNER_CACHE[key]


_WCACHE = {}


def _prepack_cached(inputs):
    """Reuse the prepacked weights when the raw weight arrays are unchanged
    (same objects, or byte-identical)."""
    names = [k for k in inputs if k != "x"]
    raw = {k: np.asarray(inputs[k]) for k in names}
    if _WCACHE:
        old = _WCACHE["raw"]
        same = all(
            old[k] is raw[k] or (
                old[k].shape == raw[k].shape and old[k].dtype == raw[k].dtype
                and np.array_equal(old[k], raw[k]))
            for k in names) if set(old) == set(raw) else False
        if same:
            return _WCACHE["shared"], _WCACHE["ln_skip"]
    shared, ln_skip = _prepack_weights(inputs)
    _WCACHE.clear()
    _WCACHE.update(raw=raw, shared=shared, ln_skip=ln_skip)
    return shared, ln_skip


def kernel(**inputs):
    x = np.asarray(inputs["x"], np.float32)
    B, N, _ = x.shape
    T = B * N // NCORES
    shared, ln_skip = _prepack_cached(inputs)
    nc = _get_compiled(T, 512, ln_skip)

    xTs = [np.ascontiguousarray(x.reshape(-1, 4)[c * T:(c + 1) * T].T)
           for c in range(NCORES)]
    xBs = [a.astype(NPBF) for a in xTs]
    try:
        runner = _get_runner(nc)
        res = runner.run(shared, {"xT": xTs, "XB": xBs})
        outs = [res["OUT"][c].T for c in range(NCORES)]
    except Exception as e:  # pragma: no cover - safety net
        import traceback
        traceback.print_exc()
        print(f"fast path failed ({e!r}); falling back to run_bass_kernel_spmd")
        in_maps = []
        for c in range(NCORES):
            m = dict(shared)
            m["xT"] = xTs[c]
            m["XB"] = xBs[c]
            in_maps.append(m)
        res = run_bass_kernel_spmd(nc, in_maps, core_ids=list(range(NCORES)))
        outs = [res.results[c]["OUT"].T for c in range(NCORES)]
    full = np.concatenate(outs, axis=0).reshape(B, N, 4).astype(np.float32)
    return full


# revision 15
# speedup vs baseline: 120.7731x; 1.0542x over previous
"""Trainium2 Bass kernel for nn_HCNetFull (dense_mlp), 8-core data parallel.

Strategy: shard the 32768 tokens across 8 NeuronCores (4096 each).
Fully feature-major bf16 dataflow: the residual stream lives in SBUF as
[128 features, 4 tiles, T tokens] and never changes layout, so there are
no 128x128 PE transposes in the steady state.  All matmuls run in bf16
(1 cyc/col).  LayerNorm statistics are computed with ones-matmuls on the
PE (contraction over the feature/partition axis), the per-token mean and
rsqrt(var) rows are broadcast back across partitions with K=1 matmuls,
and the normalization applies on the DVE.  LN1 of layers >=1 is skipped:
its input is the previous LN2 output (already zero-mean/unit-var, and
n1_g=1, n1_b=0), so LN1 is identity up to O(eps)=1e-5.

The per-group outer-product mixing uses the modular-shift symmetric
factorization (40 products x_i*x_{(i+d)%8} per group, d=0..4).  The 2560
outer-product features are produced directly in feature-major form by
two 0/1 permutation matmuls (R0, R1 - 5 shared 128x128 lhsT blocks each)
followed by an elementwise product; the 2560->512 contraction uses 5
shared GEOS lhsT blocks (the block pattern repeats with period 5).

Host side: the jitted shard_map executable and the device-resident
weight arrays are cached across kernel() calls (the axon tunnel moves
~40MB/s, so re-shipping 130MB of replicated weights per call dominates
wall time otherwise).  Weights and x are revalidated against the cached
host copies each call (object identity, then byte equality).
"""

import numpy as np
from contextlib import ExitStack

import concourse.bass as bass
import concourse.tile as tile
from concourse import bacc, mybir
from concourse.bass_utils import run_bass_kernel_spmd
from concourse.masks import make_identity

F32 = mybir.dt.float32
BF16 = mybir.dt.bfloat16
NPBF = mybir.dt.np(BF16)
D, DD, L, GS, G, P = 512, 1024, 8, 8, 64, 128
NCORES = 8
AF = mybir.ActivationFunctionType
ALU = None


def _alu():
    global ALU
    if ALU is None:
        ALU = mybir.AluOpType
    return ALU


def build_nc(T, CH, ln_skip, reps=1):
    alu = _alu()
    NCH = T // CH
    TS = CH // P

    nc = bacc.Bacc("TRN2", target_bir_lowering=False, debug=False)

    def din(name, shape, dt=BF16):
        return nc.dram_tensor(name, list(shape), dt, kind="ExternalInput")

    xT = din("xT", (4, T), F32)
    XB = din("XB", (4, T))
    W1 = din("W1", (L, D, DD)); B1 = din("B1", (L, P, 8), F32)
    W2 = din("W2", (L, DD, D)); B2 = din("B2", (L, P, 4), F32)
    GEOS = din("GEOS", (L, 5, P, P)); GBT = din("GBT", (L, P, 1), F32)
    R0C = din("R0C", (5, P, P)); R1C = din("R1C", (5, P, P))
    WIN = din("WIN", (4, D)); BIN = din("BIN", (P, 4), F32)
    GPV = din("GPV", (4, P, 16)); BPV = din("BPV", (16, 1), F32)
    GIW = din("GIW", (G, D)); BGI = din("BGI", (P, 4), F32)
    PI1 = din("PI1", (D, D)); BP1 = din("BP1", (P, 4), F32)
    PI2 = din("PI2", (D, D)); BP2 = din("BP2", (P, 4), F32)
    OW = din("OW", (4, P, 4)); OB = din("OB", (4, 1), F32)
    if not ln_skip:
        G2F = din("G2F", (L, P, 4), F32); B2F = din("B2F", (L, P, 4), F32)
    OUT = nc.dram_tensor("OUT", [4, T], F32, kind="ExternalOutput")

    with tile.TileContext(nc) as tc, ExitStack() as _px:
        cst = _px.enter_context(tc.tile_pool(name="cst", bufs=1))
        wl = _px.enter_context(tc.tile_pool(name="wl", bufs=2))
        hp = _px.enter_context(tc.tile_pool(name="hp", bufs=1))
        xfp = _px.enter_context(tc.tile_pool(name="xfp", bufs=2))
        z1p = _px.enter_context(tc.tile_pool(name="z1p", bufs=2))
        z2p = _px.enter_context(tc.tile_pool(name="z2p", bufs=2))
        ytp = _px.enter_context(tc.tile_pool(name="ytp", bufs=2))
        pp = _px.enter_context(tc.tile_pool(name="pp", bufs=1))
        gfp = _px.enter_context(tc.tile_pool(name="gfp", bufs=2))
        sqp = _px.enter_context(tc.tile_pool(name="sqp", bufs=2))
        sm = _px.enter_context(tc.tile_pool(name="sm", bufs=2))
        st = _px.enter_context(tc.tile_pool(name="st", bufs=2))
        ps_mm = _px.enter_context(tc.tile_pool(name="ps_mm", bufs=2, space="PSUM"))
        ps_x0 = _px.enter_context(tc.tile_pool(name="ps_x0", bufs=2, space="PSUM"))
        ps_x1 = _px.enter_context(tc.tile_pool(name="ps_x1", bufs=1, space="PSUM"))
        ps_g = _px.enter_context(tc.tile_pool(name="ps_g", bufs=1, space="PSUM"))
        ps_t2 = _px.enter_context(tc.tile_pool(name="ps_t2", bufs=1, space="PSUM"))

        ident = cst.tile([P, P], BF16)
        make_identity(nc, ident)
        eps_t = cst.tile([P, 1], F32)
        nc.vector.memset(eps_t, 1e-5)
        onesD = cst.tile([P, 1], BF16)
        nc.vector.memset(onesD, 1.0 / D)   # 2^-9, exact in bf16
        ones1 = cst.tile([1, P], BF16)
        nc.vector.memset(ones1, 1.0)
        win_sb = cst.tile([4, 4, P], BF16)
        nc.sync.dma_start(out=win_sb, in_=WIN[:, :].rearrange("p (mt c) -> p mt c", c=P))
        bin_sb = cst.tile([P, 4], F32)
        nc.sync.dma_start(out=bin_sb, in_=BIN[:, :])
        gpv_sb = cst.tile([P, 4, 16], BF16)
        nc.sync.dma_start(out=gpv_sb, in_=GPV[:, :, :].rearrange("kt p c -> p kt c"))
        bpv_sb = cst.tile([16, 1], F32)
        nc.sync.dma_start(out=bpv_sb, in_=BPV[:, :])
        giw_sb = cst.tile([G, D], BF16)
        nc.sync.dma_start(out=giw_sb, in_=GIW[:, :])
        bgi_sb = cst.tile([P, 4], F32)
        nc.sync.dma_start(out=bgi_sb, in_=BGI[:, :])
        pi1_sb = cst.tile([P, 4, D], BF16)
        nc.sync.dma_start(out=pi1_sb, in_=PI1[:, :].rearrange("(kt p) c -> p kt c", p=P))
        pi2_sb = cst.tile([P, 4, D], BF16)
        nc.sync.dma_start(out=pi2_sb, in_=PI2[:, :].rearrange("(kt p) c -> p kt c", p=P))
        bp1_sb = cst.tile([P, 4], F32)
        nc.sync.dma_start(out=bp1_sb, in_=BP1[:, :])
        bp2_sb = cst.tile([P, 4], F32)
        nc.sync.dma_start(out=bp2_sb, in_=BP2[:, :])
        ow_sb = cst.tile([P, 4, 4], BF16)
        nc.sync.dma_start(out=ow_sb, in_=OW[:, :, :].rearrange("kt p c -> p kt c"))
        ob_sb = cst.tile([4, 1], F32)
        nc.sync.dma_start(out=ob_sb, in_=OB[:, :])
        r0_sb = cst.tile([P, 5, P], BF16)
        nc.sync.dma_start(out=r0_sb, in_=R0C[:, :, :].rearrange("r p c -> p r c"))
        r1_sb = cst.tile([P, 5, P], BF16)
        nc.sync.dma_start(out=r1_sb, in_=R1C[:, :, :].rearrange("r p c -> p r c"))

        # persistent feature-major residual: [128 feat, 4 tiles, T tokens]
        hT = hp.tile([P, 4, T], BF16)

        def ln_fm(src, dst):
            """Feature-major LayerNorm: src/dst [P, 4, CH] bf16 SBUF."""
            sq = sqp.tile([P, 4, CH], BF16, tag="sq")
            nc.gpsimd.tensor_mul(out=sq, in0=src, in1=src)
            mq = ps_t2.tile([P, CH], F32, tag="tp2f")
            for kt in range(4):
                nc.tensor.matmul(mq[0:1, :], onesD, src[:, kt, :],
                                 start=(kt == 0), stop=(kt == 3))
            qg = ps_g.tile([P, CH], F32, tag="gps")
            for kt in range(4):
                nc.tensor.matmul(qg[0:1, :], onesD, sq[:, kt, :],
                                 start=(kt == 0), stop=(kt == 3))
            mrow = st.tile([1, CH], F32, tag="mrow")
            nc.scalar.copy(out=mrow, in_=mq[0:1, :])
            qrow = st.tile([1, CH], F32, tag="qrow")
            nc.scalar.copy(out=qrow, in_=qg[0:1, :])
            msq = st.tile([1, CH], F32, tag="msq")
            nc.vector.tensor_mul(out=msq, in0=mrow, in1=mrow)
            var = st.tile([1, CH], F32, tag="var")
            nc.vector.tensor_sub(out=var, in0=qrow, in1=msq)
            sd = st.tile([1, CH], F32, tag="sd")
            nc.scalar.activation(out=sd, in_=var, func=AF.Sqrt, bias=eps_t[0:1])
            rs_f = st.tile([1, CH], F32, tag="rs_f")
            nc.vector.reciprocal(out=rs_f, in_=sd)
            rs_bf = st.tile([1, CH], BF16, tag="rs_bf")
            nc.gpsimd.tensor_copy(out=rs_bf, in_=rs_f)
            m_bf = st.tile([1, CH], BF16, tag="m_bf")
            nc.gpsimd.tensor_copy(out=m_bf, in_=mrow)
            mB = ps_x0.tile([P, CH], F32, tag="xb0")
            nc.tensor.matmul(mB, ones1, m_bf, start=True, stop=True)
            rB = ps_x1.tile([P, CH], F32, tag="xb1")
            nc.tensor.matmul(rB, ones1, rs_bf, start=True, stop=True)
            for kt in range(4):
                tctr = sm.tile([P, CH], BF16, tag="tctr")
                nc.vector.tensor_sub(out=tctr, in0=src[:, kt, :], in1=mB)
                nc.vector.tensor_mul(out=dst[:, kt, :], in0=tctr, in1=rB)

        for _rep in range(reps):
            # ---- transformer layers (layer 0 fuses the input projection) ----
            for l in range(L):
                w1t = wl.tile([P, 4, DD], BF16, tag="w1")
                nc.sync.dma_start(out=w1t, in_=W1[l].rearrange("(kt p) c -> p kt c", p=P))
                w2t = wl.tile([P, 8, D], BF16, tag="w2")
                nc.sync.dma_start(out=w2t, in_=W2[l].rearrange("(kt p) c -> p kt c", p=P))
                geot = wl.tile([P, 5, P], BF16, tag="geo")
                nc.sync.dma_start(out=geot, in_=GEOS[l].rearrange("r p c -> p r c"))
                b1t = wl.tile([P, 8], F32, tag="b1")
                nc.sync.dma_start(out=b1t, in_=B1[l])
                b2t = wl.tile([P, 4], F32, tag="b2")
                nc.sync.dma_start(out=b2t, in_=B2[l])
                gbt = wl.tile([P, 1], F32, tag="gb")
                nc.sync.dma_start(out=gbt, in_=GBT[l])
                if not ln_skip:
                    g2t = wl.tile([P, 4], F32, tag="g2")
                    nc.sync.dma_start(out=g2t, in_=G2F[l])
                    b2rt = wl.tile([P, 4], F32, tag="b2r")
                    nc.sync.dma_start(out=b2rt, in_=B2F[l])

                for c in range(NCH):
                    sl = slice(c * CH, (c + 1) * CH)
                    hc = hT[:, :, sl]
                    if l == 0:
                        # fused input projection: h0 = x @ Win + bin
                        xcb = sm.tile([4, CH], BF16, tag="xcb")
                        nc.sync.dma_start(out=xcb, in_=XB[:, sl])
                        for mt in range(4):
                            pm = ps_mm.tile([P, CH], F32, tag="mm")
                            nc.tensor.matmul(pm, win_sb[:, mt, :], xcb,
                                             start=True, stop=True)
                            nc.scalar.activation(out=hc[:, mt, :], in_=pm,
                                                 func=AF.Identity,
                                                 bias=bin_sb[:, mt:mt + 1])
                    if l == 0 or not ln_skip:
                        xf = xfp.tile([P, 4, CH], BF16, tag="xf")
                        ln_fm(hc, xf)
                    else:
                        xf = hc
                    # fc1 + gelu
                    z1 = z1p.tile([P, 8, CH], BF16, tag="z1")
                    for mt in range(8):
                        pm = ps_mm.tile([P, CH], F32, tag="mm")
                        for kt in range(4):
                            nc.tensor.matmul(pm, w1t[:, kt, mt * P:(mt + 1) * P],
                                             xf[:, kt, :], start=(kt == 0), stop=(kt == 3))
                        nc.scalar.activation(out=z1[:, mt, :], in_=pm, func=AF.Gelu,
                                             bias=b1t[:, mt:mt + 1])
                    # fc2
                    z2 = z2p.tile([P, 4, CH], BF16, tag="z2")
                    for ft in range(4):
                        pm = ps_mm.tile([P, CH], F32, tag="mm")
                        for kt in range(8):
                            nc.tensor.matmul(pm, w2t[:, kt, ft * P:(ft + 1) * P],
                                             z1[:, kt, :], start=(kt == 0), stop=(kt == 7))
                        nc.scalar.activation(out=z2[:, ft, :], in_=pm, func=AF.Identity,
                                             bias=b2t[:, ft:ft + 1])
                    # residual (feature-major, no transpose)
                    yT = ytp.tile([P, 4, CH], BF16, tag="yT")
                    nc.gpsimd.tensor_add(out=yT, in0=z2, in1=hc)
                    # outer-product features via permutation matmuls
                    PT = pp.tile([P, 20, CH], BF16, tag="PT")
                    for r in range(5):
                        for m in range(4):
                            xb0 = ps_x0.tile([P, CH], F32, tag="xb0")
                            nc.tensor.matmul(xb0, r0_sb[:, r, :], yT[:, m, :],
                                             start=True, stop=True)
                            xb1 = ps_x1.tile([P, CH], F32, tag="xb1")
                            nc.tensor.matmul(xb1, r1_sb[:, r, :], yT[:, m, :],
                                             start=True, stop=True)
                            x0s = sm.tile([P, CH], BF16, tag="x0s")
                            nc.scalar.copy(out=x0s, in_=xb0)
                            nc.vector.tensor_mul(out=PT[:, 5 * m + r, :],
                                                 in0=x0s, in1=xb1)
                    gf = gfp.tile([P, 4, CH], BF16, tag="gf")
                    for m in range(4):
                        pg = ps_g.tile([P, CH], F32, tag="gps")
                        for r in range(5):
                            nc.tensor.matmul(pg, geot[:, r, :],
                                             PT[:, 5 * m + r, :],
                                             start=(r == 0), stop=(r == 4))
                        nc.scalar.activation(out=gf[:, m, :], in_=pg,
                                             func=AF.Identity, bias=gbt[:, 0:1])
                    # y2 = y + 0.1*geo ; LN2 -> h (in place)
                    nc.vector.scalar_tensor_tensor(
                        out=yT, in0=gf, scalar=0.1, in1=yT,
                        op0=alu.mult, op1=alu.add)
                    ln_fm(yT, hc)
                    if not ln_skip:
                        for kt in range(4):
                            nc.vector.tensor_scalar(
                                out=hc[:, kt, :], in0=hc[:, kt, :],
                                scalar1=g2t[:, kt:kt + 1], scalar2=b2rt[:, kt:kt + 1],
                                op0=alu.mult, op1=alu.add)

            # ---- GeometricInteraction ----
            for c in range(NCH):
                sl = slice(c * CH, (c + 1) * CH)
                hc = hT[:, :, sl]
                pv = ps_t2.tile([P, CH], F32, tag="tp2f")
                for kt in range(4):
                    nc.tensor.matmul(pv[:16, :], gpv_sb[:, kt, :], hc[:, kt, :],
                                     start=(kt == 0), stop=(kt == 3))
                pvsb = sm.tile([16, CH], BF16, tag="pvsb")
                nc.scalar.activation(out=pvsb, in_=pv[:16, :], func=AF.Identity,
                                     bias=bpv_sb)
                ivT = sm.tile([G, TS, P], BF16, tag="ivT")
                for ts in range(TS):
                    tp2 = ps_t2.tile([P, CH], BF16, tag="tp2")
                    nc.tensor.transpose(tp2[:, 0:16], pvsb[:, ts * P:(ts + 1) * P],
                                        ident[:16, :16])
                    pvt = sm.tile([P, 16], BF16, tag="pvt")
                    nc.vector.tensor_copy(out=pvt, in_=tp2[:, 0:16])
                    iv = sm.tile([P, GS, GS], BF16, tag="iv")
                    nc.vector.tensor_mul(
                        out=iv,
                        in0=pvt[:, 0:8].unsqueeze(2).to_broadcast((P, GS, GS)),
                        in1=pvt[:, 8:16].unsqueeze(1).to_broadcast((P, GS, GS)))
                    tp3 = ps_t2.tile([P, CH], BF16, tag="tp2")
                    nc.tensor.transpose(tp3[:G, 0:P], iv.rearrange("p a b -> p (a b)"),
                                        ident)
                    nc.vector.tensor_copy(out=ivT[:, ts, :], in_=tp3[:G, 0:P])
                itf = z2p.tile([P, 4, CH], BF16, tag="z2")
                for ft in range(4):
                    pm = ps_mm.tile([P, CH], F32, tag="mm")
                    nc.tensor.matmul(pm, giw_sb[:, ft * P:(ft + 1) * P],
                                     ivT.rearrange("p ts c -> p (ts c)"),
                                     start=True, stop=True)
                    nc.scalar.activation(out=itf[:, ft, :], in_=pm, func=AF.Identity,
                                         bias=bgi_sb[:, ft:ft + 1])
                yT = ytp.tile([P, 4, CH], BF16, tag="yT")
                nc.gpsimd.tensor_add(out=yT, in0=itf, in1=hc)
                ln_fm(yT, hc)

            # ---- particle MLP + output ----
            for c in range(NCH):
                sl = slice(c * CH, (c + 1) * CH)
                hc = hT[:, :, sl]
                z1 = z1p.tile([P, 8, CH], BF16, tag="z1")
                for mt in range(4):
                    pm = ps_mm.tile([P, CH], F32, tag="mm")
                    for kt in range(4):
                        nc.tensor.matmul(pm, pi1_sb[:, kt, mt * P:(mt + 1) * P],
                                         hc[:, kt, :], start=(kt == 0), stop=(kt == 3))
                    nc.scalar.activation(out=z1[:, mt, :], in_=pm, func=AF.Gelu,
                                         bias=bp1_sb[:, mt:mt + 1])
                z2 = z2p.tile([P, 4, CH], BF16, tag="z2")
                for ft in range(4):
                    pm = ps_mm.tile([P, CH], F32, tag="mm")
                    for kt in range(4):
                        nc.tensor.matmul(pm, pi2_sb[:, kt, ft * P:(ft + 1) * P],
                                         z1[:, kt, :], start=(kt == 0), stop=(kt == 3))
                    nc.scalar.activation(out=z2[:, ft, :], in_=pm, func=AF.Identity,
                                         bias=bp2_sb[:, ft:ft + 1])
                po = ps_t2.tile([P, CH], F32, tag="tp2f")
                for kt in range(4):
                    nc.tensor.matmul(po[:4, :], ow_sb[:, kt, :], z2[:, kt, :],
                                     start=(kt == 0), stop=(kt == 3))
                xc = sm.tile([4, CH], F32, tag="xc")
                nc.sync.dma_start(out=xc, in_=xT[:, sl])
                osb = sm.tile([4, CH], F32, tag="osb")
                nc.vector.scalar_tensor_tensor(
                    out=osb, in0=po[:4, :], scalar=ob_sb, in1=xc,
                    op0=alu.add, op1=alu.add)
                nc.sync.dma_start(out=OUT[:, sl], in_=osb)

    nc.compile()
    return nc


def _prepack_weights(inputs):
    """Host-side weight packing. Returns (shared dict, ln_skip)."""
    f = lambda a: np.ascontiguousarray(np.asarray(a, np.float32))
    in_w, in_b = f(inputs["in_w"]), f(inputs["in_b"])
    fc1_w, fc1_b = f(inputs["fc1_w"]), f(inputs["fc1_b"])
    fc2_w, fc2_b = f(inputs["fc2_w"]), f(inputs["fc2_b"])
    geo_w, geo_b = f(inputs["geo_w"]), f(inputs["geo_b"])
    n1_g, n1_b = f(inputs["n1_g"]), f(inputs["n1_b"])
    n2_g, n2_b = f(inputs["n2_g"]), f(inputs["n2_b"])

    W1 = (n1_g[:, :, None] * fc1_w).astype(NPBF)
    b1full = fc1_b + np.einsum("ld,lde->le", n1_b, fc1_w)
    B1 = b1full.reshape(L, 8, P).transpose(0, 2, 1).copy()
    W2 = fc2_w.astype(NPBF)
    B2 = fc2_b.reshape(L, 4, P).transpose(0, 2, 1).copy()

    # modular-shift symmetric geo weights: w_mod[d,i,k], pairs (i,(i+d)%8)
    gw3 = geo_w.reshape(L, 8, 8, 8)
    wmod = np.zeros((L, 5, 8, 8), np.float32)
    ii = np.arange(8)
    for d in range(5):
        jj = (ii + d) % 8
        if d == 0:
            wmod[:, d] = gw3[:, ii, ii, :]
        elif d == 4:
            wmod[:, d] = 0.5 * (gw3[:, ii, jj, :] + gw3[:, jj, ii, :])
        else:
            wmod[:, d] = gw3[:, ii, jj, :] + gw3[:, jj, ii, :]
    # block matrix for one 128-col output block (16 groups); chunks repeat
    # with period 5 across the 20 feature chunks.
    blk = np.zeros((L, 16, 5, 8, 16, 8), np.float32)
    for g in range(16):
        blk[:, g, :, :, g, :] = wmod
    GEOS = blk.reshape(L, 640, 128).reshape(L, 5, 128, 128).astype(NPBF)
    GBT = np.tile(geo_b, (1, 16)).reshape(L, P, 1).astype(np.float32)

    # permutation matrices for feature-major outer products:
    # dest p (in chunk r): gg=(p+128r)//40, u=(p+128r)%40, d=u//8, i=u%8
    # R0 source q = 8*gg + i ; R1 source q = 8*gg + (i+d)%8
    R0C = np.zeros((5, P, P), np.float32)
    R1C = np.zeros((5, P, P), np.float32)
    for r in range(5):
        for pcol in range(P):
            frel = pcol + 128 * r
            gg, u = frel // 40, frel % 40
            dd, i = u // 8, u % 8
            if gg < 16:
                R0C[r, 8 * gg + i, pcol] = 1.0
                R1C[r, 8 * gg + (i + dd) % 8, pcol] = 1.0

    BIN = in_b.reshape(4, P).T.copy()
    GPV = np.concatenate(
        [f(inputs["gi_pos_w"]), f(inputs["gi_vel_w"])], axis=1
    ).reshape(4, P, 16).astype(NPBF)
    BPV = np.concatenate([f(inputs["gi_pos_b"]), f(inputs["gi_vel_b"])])[:, None]
    GIW = f(inputs["gi_int_w"]).astype(NPBF)
    BGI = f(inputs["gi_int_b"]).reshape(4, P).T.copy()
    gn_g, gn_b = f(inputs["gi_n_g"]), f(inputs["gi_n_b"])
    PI1 = (gn_g[:, None] * f(inputs["pi1_w"])).astype(NPBF)
    bp1full = f(inputs["pi1_b"]) + gn_b @ f(inputs["pi1_w"])
    BP1 = bp1full.reshape(4, P).T.copy()
    PI2 = f(inputs["pi2_w"]).astype(NPBF)
    BP2 = f(inputs["pi2_b"]).reshape(4, P).T.copy()
    OW = f(inputs["out_w"]).reshape(4, P, 4).astype(NPBF)
    OB = f(inputs["out_b"])[:, None]

    ln_skip = (np.all(n1_g == 1.0) and np.all(n1_b == 0.0)
               and np.all(n2_g == 1.0) and np.all(n2_b == 0.0))
    shared = dict(W1=W1, B1=B1, W2=W2, B2=B2, GEOS=GEOS, GBT=GBT,
                  R0C=R0C.astype(NPBF), R1C=R1C.astype(NPBF),
                  WIN=in_w.astype(NPBF), BIN=BIN, GPV=GPV, BPV=BPV,
                  GIW=GIW, BGI=BGI, PI1=PI1, BP1=BP1, PI2=PI2, BP2=BP2,
                  OW=OW, OB=OB)
    if not ln_skip:
        shared["G2F"] = np.ascontiguousarray(
            n2_g.reshape(L, 4, P).transpose(0, 2, 1), np.float32)
        shared["B2F"] = np.ascontiguousarray(
            n2_b.reshape(L, 4, P).transpose(0, 2, 1), np.float32)
    shared = {k: np.ascontiguousarray(v) for k, v in shared.items()}
    return shared, ln_skip


_NC_CACHE = {}


def _get_compiled(T, CH, ln_skip, reps=1):
    key = (T, CH, ln_skip, reps)
    if key not in _NC_CACHE:
        _NC_CACHE[key] = build_nc(T, CH, ln_skip, reps)
    return _NC_CACHE[key]


class _FastRunner:
    """Caches the jitted shard_map executable and device-resident inputs.

    Weight inputs are revalidated against the cached host copies on every
    call (object identity first, then byte equality); x-derived tensors are
    byte-compared and re-shipped only when they change.
    """

    def __init__(self, nc, n_cores):
        import jax
        from jax.sharding import Mesh, PartitionSpec, NamedSharding
        from jax.experimental.shard_map import shard_map
        from concourse.bass2jax import (_bass_exec_p, install_neuronx_cc_hook,
                                        partition_id_tensor)
        install_neuronx_cc_hook()
        self.jax = jax
        self.nc = nc
        self.n_cores = n_cores
        partition_name = (nc.partition_id_tensor.name
                          if nc.partition_id_tensor else None)
        in_names, out_names, out_avals, zero_outs = [], [], [], []
        for alloc in nc.m.functions[0].allocations:
            if not isinstance(alloc, mybir.MemoryLocationSet):
                continue
            name = alloc.memorylocations[0].name
            if alloc.kind == "ExternalInput":
                if name != partition_name:
                    in_names.append(name)
            elif alloc.kind == "ExternalOutput":
                out_names.append(name)
                shape = tuple(alloc.tensor_shape)
                dtype = mybir.dt.np(alloc.dtype)
                out_avals.append(jax.core.ShapedArray(shape, dtype))
                zero_outs.append(np.zeros(shape, dtype))
        self.in_names = in_names
        self.out_names = out_names
        self.out_avals = out_avals
        self.zero_outs = zero_outs
        n_params = len(in_names)
        n_outs = len(out_avals)
        in_names_full = in_names + out_names + (
            [partition_name] if partition_name else [])

        def _body(*args):
            operands = list(args)
            if partition_name is not None:
                operands.append(partition_id_tensor())
            outs = _bass_exec_p.bind(
                *operands, out_avals=tuple(out_avals),
                in_names=tuple(in_names_full), out_names=tuple(out_names),
                lowering_input_output_aliases=(), sim_require_finite=True,
                sim_require_nnan=True, nc=nc)
            return tuple(outs)

        devices = jax.devices()[:n_cores]
        self.mesh = Mesh(np.asarray(devices), ("core",))
        self.sharding = NamedSharding(self.mesh, PartitionSpec("core"))
        in_specs = (PartitionSpec("core"),) * (n_params + n_outs)
        out_specs = (PartitionSpec("core"),) * len(out_names)
        donate = tuple(range(n_params, n_params + n_outs))
        self.sharded = jax.jit(
            shard_map(_body, mesh=self.mesh, in_specs=in_specs,
                      out_specs=out_specs, check_rep=False),
            donate_argnums=donate, keep_unused=True)
        self._host_cache = {}   # name -> host np array
        self._dev_cache = {}    # name -> device array

    def run(self, shared, volatile_maps):
        """shared: dict name->array (replicated); volatile_maps: name->list of
        per-core arrays (re-validated by byte equality each call)."""
        jax = self.jax
        n = self.n_cores
        args = []
        for name in self.in_names:
            if name in volatile_maps:
                cat = np.concatenate(
                    [np.asarray(a) for a in volatile_maps[name]], axis=0)
                cached = self._host_cache.get(name)
                if (cached is not None and cached.shape == cat.shape
                        and cached.dtype == cat.dtype
                        and np.array_equal(cached, cat)):
                    args.append(self._dev_cache[name])
                else:
                    dev = jax.device_put(cat, self.sharding)
                    self._host_cache[name] = cat
                    self._dev_cache[name] = dev
                    args.append(dev)
                continue
            arr = shared[name]
            cached = self._host_cache.get(name)
            if cached is not None and cached is arr:
                args.append(self._dev_cache[name])
                continue
            if (cached is not None and cached.shape == arr.shape
                    and cached.dtype == arr.dtype
                    and np.array_equal(cached, arr)):
                self._host_cache[name] = arr
                args.append(self._dev_cache[name])
                continue
            cat = np.concatenate([arr] * n, axis=0)
            dev = jax.device_put(cat, self.sharding)
            self._host_cache[name] = arr
            self._dev_cache[name] = dev
            args.append(dev)
        for z in self.zero_outs:
            args.append(np.zeros((n * z.shape[0], *z.shape[1:]), z.dtype))
        out_arrs = self.sharded(*args)
        outs = []
        for i, name in enumerate(self.out_names):
            a = np.asarray(out_arrs[i]).reshape(n, *self.out_avals[i].shape)
            outs.append(a)
        return {name: outs[i] for i, name in enumerate(self.out_names)}


_RUNNER_CACHE = {}


def _get_runner(nc):
    key = id(nc)
    if key not in _RUNNER_CACHE:
        _RUNNER_CACHE[key] = _FastRunner(nc, NCORES)
    return _RUNNER_CACHE[key]


_WCACHE = {}


def _prepack_cached(inputs):
    """Reuse the prepacked weights when the raw weight arrays are unchanged
    (same objects, or byte-identical)."""
    names = [k for k in inputs if k != "x"]
    raw = {k: np.asarray(inputs[k]) for k in names}
    if _WCACHE:
        old = _WCACHE["raw"]
        same = all(
            old[k] is raw[k] or (
                old[k].shape == raw[k].shape and old[k].dtype == raw[k].dtype
                and np.array_equal(old[k], raw[k]))
            for k in names) if set(old) == set(raw) else False
        if same:
            return _WCACHE["shared"], _WCACHE["ln_skip"]
    shared, ln_skip = _prepack_weights(inputs)
    _WCACHE.clear()
    _WCACHE.update(raw=raw, shared=shared, ln_skip=ln_skip)
    return shared, ln_skip


def kernel(**inputs):
    x = np.asarray(inputs["x"], np.float32)
    B, N, _ = x.shape
    T = B * N // NCORES
    shared, ln_skip = _prepack_cached(inputs)
    nc = _get_compiled(T, 512, ln_skip)

    xTs = [np.ascontiguousarray(x.reshape(-1, 4)[c * T:(c + 1) * T].T)
           for c in range(NCORES)]
    xBs = [a.astype(NPBF) for a in xTs]
    try:
        runner = _get_runner(nc)
        res = runner.run(shared, {"xT": xTs, "XB": xBs})
        outs = [res["OUT"][c].T for c in range(NCORES)]
    except Exception as e:  # pragma: no cover - safety net
        import traceback
        traceback.print_exc()
        print(f"fast path failed ({e!r}); falling back to run_bass_kernel_spmd")
        in_maps = []
        for c in range(NCORES):
            m = dict(shared)
            m["xT"] = xTs[c]
            m["XB"] = xBs[c]
            in_maps.append(m)
        res = run_bass_kernel_spmd(nc, in_maps, core_ids=list(range(NCORES)))
        outs = [res.results[c]["OUT"].T for c in range(NCORES)]
    full = np.concatenate(outs, axis=0).reshape(B, N, 4).astype(np.float32)
    return full
